# revision 1
# baseline (speedup 1.0000x reference)
"""Trainium2 Bass kernel for nn_EventWarping (contrast-maximization event warping loss).

Strategy (data-parallel over batch, one batch element per NeuronCore):
  - flow gather at integer event pixels: indirect DMA (per-event 8B row gather)
  - warped coords wy/wx per event per tref: bulk DVE/ACT math on [128, N/128] tiles
  - the 4 scatter-add histograms (iwe / iwe_ts x pol0 / all) are computed as
    sums of rank-1 outer products of bilinear "hat" row/col vectors:
        hatY[e, y] = relu(1 - |y - wy_e|)   (nonzero at floor(wy), floor(wy)+1)
        hatX[e, x] = relu(1 - |x - wx_e|)
        IWE = sum_e s_e * hatY_e (x) hatX_e  ==  (hatY*s)^T @ hatX  (PSUM accum)
    Out-of-bounds corners drop out automatically (hat is 0 on the [0,H)x[0,W) grid).
  - per-tref loss terms and charbonnier smoothness reduced on device to
    per-partition partial sums; host sums 8 cores' partials.
"""

import os
import sys

import numpy as np

sys.path.insert(0, "/opt/trn_rl_repo")

import concourse.bass as bass
import concourse.bacc as bacc
import concourse.tile as tile
from concourse import mybir
from concourse.alu_op_type import AluOpType as Alu

F32 = mybir.dt.float32
BF16 = mybir.dt.bfloat16
F16 = mybir.dt.float16
FP8 = mybir.dt.float8e4
I32 = mybir.dt.int32
AF = mybir.ActivationFunctionType
DR = mybir.MatmulPerfMode.DoubleRow

H, W = 256, 336
SCALE = 336.0
REG_WEIGHT = 0.001
B = 8
N_FULL = 262144
P = 128

# output column layout of the [128, 16] partials tensor
LOSS_COLS = list(range(8))     # 2 trefs x 2 pols x 2 y-halves
SMOOTH_COLS = list(range(8, 14))  # 2 ch x {dxA, dxB, dy}


def build_nc(n_events=N_FULL, group=4, y_on_act=True, mm_dt=BF16, dbg=False, gstage=4):
    """Build the SPMD Bass program for one core / one batch element."""
    cols = n_events // P  # events laid out as [128, cols], chunk = one column
    n_chunks = cols
    assert cols % group == 0

    nc = bacc.Bacc("TRN2", target_bir_lowering=False, debug=False, num_devices=8)

    ev_t = nc.declare_dram_parameter("ev_t", [4, n_events], F32, isOutput=False)
    flow_i = nc.declare_dram_parameter("flow_i", [H * W, 2], F32, isOutput=False)
    flow_r = nc.declare_dram_parameter("flow_r", [2, H, W], F32, isOutput=False)
    out = nc.declare_dram_parameter("out", [P, 16], F32, isOutput=True)
    dbg_t = (
        nc.declare_dram_parameter("dbg", [P, 8 * W], F32, isOutput=True) if dbg else None
    )

    with tile.TileContext(nc) as tc:
        with (
            tc.tile_pool(name="persist", bufs=1) as persist,
            tc.tile_pool(name="tref", bufs=1) as trefp,
            tc.tile_pool(name="grp", bufs=4) as grp,
            tc.tile_pool(name="small", bufs=1) as small,
            tc.tile_pool(name="psum", bufs=1, space="PSUM") as psum,
        ):
            # ---------------- constants ----------------
            iota_i = small.tile([P, W], I32, tag="iota_i")
            nc.gpsimd.iota(iota_i[:], pattern=[[1, W]], base=0, channel_multiplier=0)
            # value (1 - k) and (k + 1) tiles for hat construction
            # f16 holds integers up to 2048 exactly, covers |values| <= 337
            yc1 = small.tile([P, H], F32, tag="yc1")
            yc2 = small.tile([P, H], F32, tag="yc2")
            xc1 = small.tile([P, W], F16, tag="xc1")
            xc2 = small.tile([P, W], F16, tag="xc2")
            nc.vector.tensor_scalar(yc1[:], iota_i[:, :H], -1.0, 1.0, Alu.mult, Alu.add)
            nc.vector.tensor_scalar(yc2[:], iota_i[:, :H], 1.0, None, Alu.add)
            nc.vector.tensor_scalar(xc1[:], iota_i[:], -1.0, 1.0, Alu.mult, Alu.add)
            nc.vector.tensor_scalar(xc2[:], iota_i[:], 1.0, None, Alu.add)

            # ---------------- output partials tile ----------------
            out_t = small.tile([P, 16], F32, tag="out_t")
            nc.vector.memset(out_t[:], 0.0)

            c1em6 = small.tile([P, 1], F32, tag="c1em6")
            nc.vector.memset(c1em6[:], 1e-6)

            # ---------------- load event scalars ----------------
            def load_row(r, tag):
            # ev_t row r [n_events] -> [128, cols]
                t = persist.tile([P, cols], F32, tag=tag)
                nc.sync.dma_start(out=t[:], in_=ev_t[r].rearrange("(p c) -> p c", p=P))
                return t

            ts_t = load_row(0, "ts")
            ys_t = load_row(1, "ys")
            xs_t = load_row(2, "xs")
            pm0_t = trefp.tile([P, cols], F32, tag="m1", name="pm0_t")
            nc.sync.dma_start(out=pm0_t[:], in_=ev_t[3].rearrange("(p c) -> p c", p=P))

            pm0b = persist.tile([P, cols], F32, tag="pm0b")
            nc.vector.tensor_copy(pm0b[:], pm0_t[:])

            # ---------------- flow gather ----------------
            # indirect_dma_start semantics on HW: one index per output
            # partition-row; out[p, :] = in.flat[idx[p, 0]*coef : ...].
            # So gather one event-column (128 events) per instruction.
            pixi = persist.tile([P, cols], I32, tag="pixi")
            nc.vector.scalar_tensor_tensor(
                pixi[:], ys_t[:], float(W), xs_t[:], Alu.mult, Alu.add
            )

            fg = persist.tile([P, 2 * cols], F32, tag="fg")
            if gstage < 2:
                nc.vector.memset(fg[:], 0.0001)
            for c in range(cols if gstage >= 2 else 0):
                nc.gpsimd.indirect_dma_start(
                    out=fg[:, 2 * c : 2 * c + 2],
                    out_offset=None,
                    in_=flow_i[:],
                    in_offset=bass.IndirectOffsetOnAxis(ap=pixi[:, c : c + 1], axis=0),
                )

            fy_t = fg[:].rearrange("p (c two) -> p c two", two=2)[:, :, 0]
            fx_t = fg[:].rearrange("p (c two) -> p c two", two=2)[:, :, 1]

            # ---------------- charbonnier smoothness ----------------
            # per channel: F [128, 672] rows (2p, 2p+1); Fs [127, 672] rows (2p+1, 2p+2)
            smooth_scratch = small.tile([P, 672], F32, tag="smooth_scratch")
            for ch in range(2):
                Fc = small.tile([P, 2 * W], F32, tag="Fc")
                nc.sync.dma_start(out=Fc[:], in_=flow_r[ch].rearrange("(p r) w -> p (r w)", r=2))
                Fs = small.tile([P - 1, 2 * W], F32, tag="Fs")
                nc.sync.dma_start(
                    out=Fs[:],
                    in_=flow_r[ch, 1 : 2 * P - 1, :].rearrange("(p r) w -> p (r w)", r=2),
                )
                # dxA: rows 2p - (2p+1); all 128 partitions
                dxa = small.tile([P, W], F32, tag="dxa")
                nc.vector.tensor_tensor(dxa[:], Fc[:, 0:W], Fc[:, W : 2 * W], Alu.subtract)
                nc.vector.scalar_tensor_tensor(
                    smooth_scratch[:, 0:W], dxa[:], 0.0, dxa[:], Alu.add, Alu.mult
                )
                nc.scalar.activation(
                    smooth_scratch[:, 0:W], smooth_scratch[:, 0:W], AF.Sqrt,
                    bias=c1em6[:], scale=1.0,
                    accum_out=out_t[:, 8 + 3 * ch : 9 + 3 * ch],
                )
                # dxB: rows (2p+1) - (2p+2); 127 partitions
                dxb = small.tile([P - 1, W], F32, tag="dxb")
                nc.vector.tensor_tensor(
                    dxb[:], Fc[: P - 1, W : 2 * W], Fs[:, W : 2 * W], Alu.subtract
                )
                nc.vector.scalar_tensor_tensor(
                    smooth_scratch[: P - 1, 0:W], dxb[:], 0.0, dxb[:], Alu.add, Alu.mult
                )
                nc.scalar.activation(
                    smooth_scratch[: P - 1, 0:W], smooth_scratch[: P - 1, 0:W], AF.Sqrt,
                    bias=c1em6[:P-1], scale=1.0,
                    accum_out=out_t[: P - 1, 9 + 3 * ch : 10 + 3 * ch],
                )
                # dy: within-row x-diffs, 2 blocks of 335 per partition
                dy = small.tile([P, 2 * (W - 1)], F32, tag="dy")
                src_a = Fc[:].rearrange("p (r w) -> p r w", r=2)[:, :, 0 : W - 1]
                src_b = Fc[:].rearrange("p (r w) -> p r w", r=2)[:, :, 1:W]
                nc.vector.tensor_tensor(dy[:].rearrange("p (r w) -> p r w", r=2), src_a, src_b, Alu.subtract)
                nc.vector.scalar_tensor_tensor(
                    smooth_scratch[:, 0 : 2 * (W - 1)], dy[:], 0.0, dy[:], Alu.add, Alu.mult
                )
                nc.scalar.activation(
                    smooth_scratch[:, 0 : 2 * (W - 1)], smooth_scratch[:, 0 : 2 * (W - 1)],
                    AF.Sqrt, bias=c1em6[:], scale=1.0,
                    accum_out=out_t[:, 10 + 3 * ch : 11 + 3 * ch],
                )

            # ---------------- per-tref pipeline ----------------
            for it, tref in enumerate((1.0, 0.0)):
                # bulk per-event math
                wy = trefp.tile([P, cols], F32, tag="wy")
                wx = trefp.tile([P, cols], F32, tag="wx")
                m1 = trefp.tile([P, cols], F32, tag="m1")
                # m1 = (ts - tref) * f; w = m1 * (-SCALE) + base
                nc.vector.scalar_tensor_tensor(m1[:], ts_t[:], float(tref), fy_t, Alu.subtract, Alu.mult)
                nc.vector.scalar_tensor_tensor(wy[:], m1[:], -SCALE, ys_t[:], Alu.mult, Alu.add)
                nc.vector.scalar_tensor_tensor(m1[:], ts_t[:], float(tref), fx_t, Alu.subtract, Alu.mult)
                nc.vector.scalar_tensor_tensor(wx[:], m1[:], -SCALE, xs_t[:], Alu.mult, Alu.add)

                if tref == 1.0:
                    tgb = ts_t
                else:
                    tgb = trefp.tile([P, cols], F32, tag="tgb")
                    nc.vector.tensor_scalar(tgb[:], ts_t[:], -1.0, 1.0, Alu.mult, Alu.add)

                if y_on_act:
                    wyn = trefp.tile([P, cols], F32, tag="wyn")
                    nc.vector.tensor_scalar(wyn[:], wy[:], -1.0, None, Alu.mult)

                # 8 persistent psum accumulators: (ALL, P0, TSALL, TSP0) x y-half
                ps = {}
                for v in ("all", "p0", "tsall", "tsp0"):
                    for h in range(2):
                        ps[(v, h)] = psum.tile([P, W], F32, tag=f"ps_{v}_{h}", name=f"ps_{v}_{h}")

                n_groups = n_chunks // group
                n_pairs = n_chunks // 2
                for g in range(n_groups):
                    c0 = g * group
                    raY = grp.tile([P, group * H], BF16, tag="raY")
                    rbY = grp.tile([P, group * H], BF16, tag="rbY")
                    raX = grp.tile([P, group * W], BF16, tag="raX")
                    rbX = grp.tile([P, group * W], BF16, tag="rbX")
                    for j in range(group):
                        c = c0 + j
                        sy = slice(j * H, (j + 1) * H)
                        sx = slice(j * W, (j + 1) * W)
                        if y_on_act:
                            nc.scalar.activation(
                                raY[:, sy], yc1[:], AF.Relu, bias=wy[:, c : c + 1], scale=1.0
                            )
                            nc.scalar.activation(
                                rbY[:, sy], yc2[:], AF.Relu, bias=wyn[:, c : c + 1], scale=1.0
                            )
                        else:
                            nc.vector.tensor_scalar(
                                raY[:, sy], yc1[:], wy[:, c : c + 1], 0.0, Alu.add, Alu.max
                            )
                            nc.vector.tensor_scalar(
                                rbY[:, sy], yc2[:], wy[:, c : c + 1], 0.0, Alu.subtract, Alu.max
                            )
                        nc.vector.tensor_scalar(
                            raX[:, sx], xc1[:], wx[:, c : c + 1], 0.0, Alu.add, Alu.max
                        )
                        nc.vector.tensor_scalar(
                            rbX[:, sx], xc2[:], wx[:, c : c + 1], 0.0, Alu.subtract, Alu.max
                        )
                    hatY = grp.tile([P, group * H], BF16, tag="hatY")
                    hatX = grp.tile([P, group * W], BF16, tag="hatX")
                    hatYtg = grp.tile([P, group * H], BF16, tag="hatYtg")
                    hatXpm = grp.tile([P, group * W], BF16, tag="hatXpm")
                    nc.vector.tensor_tensor(hatY[:], raY[:], rbY[:], Alu.min)
                    nc.vector.tensor_tensor(hatX[:], raX[:], rbX[:], Alu.min)
                    # per-column AP-scalar multiplies stay on the DVE
                    # 2-elem/cycle path; stride-0 broadcasts do not
                    for j in range(group):
                        c = c0 + j
                        sy = slice(j * H, (j + 1) * H)
                        sx = slice(j * W, (j + 1) * W)
                        nc.vector.tensor_scalar(
                            hatYtg[:, sy], hatY[:, sy], tgb[:, c : c + 1], None, Alu.mult
                        )
                        nc.vector.tensor_scalar(
                            hatXpm[:, sx], hatX[:, sx], pm0b[:, c : c + 1], None, Alu.mult
                        )
                    for j in range(group):
                        c = c0 + j
                        start = c == 0
                        stop = c == n_chunks - 1
                        sx = slice(j * W, (j + 1) * W)
                        for h in range(2):
                            sh = slice(j * H + h * P, j * H + (h + 1) * P)
                            for v, lhs, rhs in (
                                ("all", hatY, hatX),
                                ("p0", hatY, hatXpm),
                                ("tsall", hatYtg, hatX),
                                ("tsp0", hatYtg, hatXpm),
                            ):
                                nc.tensor.matmul(
                                    ps[(v, h)][:],
                                    lhsT=lhs[:, sh],
                                    rhs=rhs[:, sx],
                                    start=start,
                                    stop=stop,
                                )

                # loss terms from accumulated images
                for h in range(2):
                    s_all = small.tile([P, W], F32, tag="s_all")
                    s_p0 = small.tile([P, W], F32, tag="s_p0")
                    t_all = small.tile([P, W], F32, tag="t_all")
                    t_p0 = small.tile([P, W], F32, tag="t_p0")
                    nc.vector.tensor_copy(s_all[:], ps[("all", h)][:])
                    nc.vector.tensor_copy(s_p0[:], ps[("p0", h)][:])
                    nc.vector.tensor_copy(t_all[:], ps[("tsall", h)][:])
                    nc.vector.tensor_copy(t_p0[:], ps[("tsp0", h)][:])
                    if dbg and it == 0:
                        for di, timg in enumerate((s_all, s_p0, t_all, t_p0)):
                            nc.sync.dma_start(
                                out=dbg_t[:, (4 * h + di) * W : (4 * h + di + 1) * W],
                                in_=timg[:],
                            )
                    # pol1 = all - pol0 (in place into s_all/t_all)
                    nc.vector.tensor_tensor(s_all[:], s_all[:], s_p0[:], Alu.subtract)
                    nc.vector.tensor_tensor(t_all[:], t_all[:], t_p0[:], Alu.subtract)
                    for pi, (S_img, T_img) in enumerate(((s_p0, t_p0), (s_all, t_all))):
                        r = small.tile([P, W], F32, tag="recip")
                        nc.vector.tensor_scalar(r[:], S_img[:], 1e-9, None, Alu.add)
                        nc.vector.reciprocal(r[:], r[:])
                        q = small.tile([P, W], F32, tag="q")
                        nc.vector.tensor_tensor(q[:], T_img[:], r[:], Alu.mult)
                        col = 4 * it + 2 * h + pi
                        nc.vector.scalar_tensor_tensor(
                            r[:], q[:], 0.0, q[:], Alu.add, Alu.mult,
                            accum_out=out_t[:, col : col + 1],
                        )

            nc.sync.dma_start(out=out[:], in_=out_t[:])

    nc.finalize()
    return nc


# ---------------------------------------------------------------------------
# host-side wrapper
# ---------------------------------------------------------------------------

_CACHED = {}


def _get_nc():
    key = "full"
    if key not in _CACHED:
        _CACHED[key] = build_nc()
    return _CACHED[key]


def prep_core_inputs(flow_b, ev_b, pm_b):
    """Per-batch-element host prep: pure re-layout (sharding), no math beyond layout."""
    n = ev_b.shape[0]
    ev_t = np.empty((4, n), dtype=np.float32)
    ev_t[0] = ev_b[:, 0]
    ev_t[1] = ev_b[:, 1]
    ev_t[2] = ev_b[:, 2]
    ev_t[3] = pm_b[:, 0]
    flow_i = np.ascontiguousarray(
        np.stack([flow_b[1].reshape(-1), flow_b[0].reshape(-1)], axis=-1)
    ).astype(np.float32)
    flow_r = np.ascontiguousarray(flow_b).astype(np.float32)
    return {"ev_t": ev_t, "flow_i": flow_i, "flow_r": flow_r}


def finish(outs):
    """Combine per-core partials into the scalar loss."""
    total = np.float64(0.0)
    for o in outs:
        o = o.astype(np.float64)
        total += o[:, LOSS_COLS].sum() + REG_WEIGHT * o[:, SMOOTH_COLS].sum()
    return np.float32(total)


def kernel(flow, event_list, pol_mask):
    from concourse.bass_utils import run_bass_kernel_spmd

    flow = np.asarray(flow)
    event_list = np.asarray(event_list)
    pol_mask = np.asarray(pol_mask)
    nc = _get_nc()
    in_maps = [
        prep_core_inputs(flow[b], event_list[b], pol_mask[b]) for b in range(B)
    ]
    res = run_bass_kernel_spmd(nc, in_maps, list(range(B)))
    outs = [res.results[b]["out"] for b in range(B)]
    return finish(outs)


if __name__ == "__main__":
    # smoke test with random data
    rng = np.random.default_rng(0)
    flow = (0.05 * rng.standard_normal((B, 2, H, W))).astype(np.float32)
    ys = rng.integers(0, H, (B, N_FULL)).astype(np.float32)
    xs = rng.integers(0, W, (B, N_FULL)).astype(np.float32)
    ts = rng.random((B, N_FULL), dtype=np.float32)
    pol = rng.integers(0, 2, (B, N_FULL))
    ev = np.stack([ts, ys, xs, pol * 2.0 - 1.0], axis=-1).astype(np.float32)
    pm = np.stack([(pol == 1), (pol == 0)], axis=-1).astype(np.float32)
    print(kernel(flow, ev, pm))



# revision 2
# speedup vs baseline: 1.0458x; 1.0458x over previous
"""Trainium2 Bass kernel for nn_EventWarping (contrast-maximization event warping loss).

Strategy (data-parallel over batch, one batch element per NeuronCore):
  - flow gather at integer event pixels: indirect DMA (per-event-column 8B row
    gather on the gpsimd software-DGE path), pipelined per 128-column block so
    hat building overlaps descriptor generation.
  - the 4 scatter-add histograms (iwe / iwe_ts x pol0 / all) are computed as
    sums of rank-1 outer products of bilinear "hat" row/col vectors, using
    fp8e4 DoubleRow matmuls (2 event-chunks contracted per pass at 0.5
    cycles/row -> 4x the bf16 streaming rate):
        hat'(d) = min(|d|,1) - 1 = -relu(1 - |d|)   (negative form, one
        tensor_scalar pass from |d|; products of two negative hats are
        positive so no sign fixup is needed)
        |d| built from d = k - w via a uint16 bitwise-and (sign-bit clear),
        both passes run in the DVE 4x perf mode.
  - PSUM has only 8 banks = one tref's 8 half-images, so each 128-column
    block runs tref=1 then tref=0 through the same banks, flushing into
    SBUF f32 accumulators between trefs.
  - per-tref loss terms and charbonnier smoothness reduced on device to
    per-partition partial sums; host sums 8 cores' partials.
"""

import sys

import numpy as np

sys.path.insert(0, "/opt/trn_rl_repo")

import concourse.bass as bass
import concourse.bacc as bacc
import concourse.tile as tile
from concourse import mybir
from concourse.alu_op_type import AluOpType as Alu

F32 = mybir.dt.float32
F16 = mybir.dt.float16
U16 = mybir.dt.uint16
FP8 = mybir.dt.float8e4
I32 = mybir.dt.int32
AF = mybir.ActivationFunctionType
DR = mybir.MatmulPerfMode.DoubleRow

H, W = 256, 336
SCALE = 336.0
REG_WEIGHT = 0.001
B = 8
N_FULL = 262144
P = 128

LOSS_COLS = list(range(8))        # 2 trefs x 2 pols x 2 y-halves
SMOOTH_COLS = list(range(8, 14))  # 2 ch x {dxA, dxB, dy}

VARIANTS = ("all", "p0", "tsall", "tsp0")


def build_nc(n_events=N_FULL, blk_cols=128, hyt_act=True, hxp_act=True):
    """Build the SPMD Bass program for one core / one batch element."""
    cols = n_events // P          # events laid out as [128, cols]
    n_blocks = cols // blk_cols
    assert blk_cols % 2 == 0
    blk_pairs = blk_cols // 2

    nc = bacc.Bacc("TRN2", target_bir_lowering=False, debug=False, num_devices=8)

    ev_t = nc.declare_dram_parameter("ev_t", [4, n_events], F32, isOutput=False)
    flow_i = nc.declare_dram_parameter("flow_i", [H * W, 2], F32, isOutput=False)
    flow_r = nc.declare_dram_parameter("flow_r", [2, H, W], F32, isOutput=False)
    out = nc.declare_dram_parameter("out", [P, 16], F32, isOutput=True)

    with tile.TileContext(nc) as tc:
        with (
            tc.tile_pool(name="persist", bufs=1) as persist,
            tc.tile_pool(name="acc", bufs=1) as accp,
            tc.tile_pool(name="blk", bufs=2) as blkp,
            tc.tile_pool(name="fgp", bufs=3) as fgp,
            tc.tile_pool(name="pair", bufs=4) as pairp,
            tc.tile_pool(name="small", bufs=1) as small,
            tc.tile_pool(name="psum", bufs=1, space="PSUM") as psum,
        ):
            # ---------------- constants ----------------
            iota_i = small.tile([P, W], I32, tag="iota_i")
            nc.gpsimd.iota(iota_i[:], pattern=[[1, W]], base=0, channel_multiplier=0)
            iota_y = small.tile([P, H], F16, tag="iota_y")
            iota_x = small.tile([P, W], F16, tag="iota_x")
            nc.vector.tensor_copy(iota_y[:], iota_i[:, :H])
            nc.vector.tensor_copy(iota_x[:], iota_i[:])

            out_t = small.tile([P, 16], F32, tag="out_t")
            nc.vector.memset(out_t[:], 0.0)
            c1em6 = small.tile([P, 1], F32, tag="c1em6")
            nc.vector.memset(c1em6[:], 1e-6)

            # ---------------- load event scalars ----------------
            def load_row(r, tag):
                t = persist.tile([P, cols], F32, tag=tag)
                nc.sync.dma_start(out=t[:], in_=ev_t[r].rearrange("(p c) -> p c", p=P))
                return t

            ts_t = load_row(0, "ts")
            ys_t = load_row(1, "ys")
            xs_t = load_row(2, "xs")
            pm_t = load_row(3, "pm")

            # per-event scalars: tg (per tref) lives in ts_t / tg0_t
            tg0_t = persist.tile([P, cols], F32, tag="tg0")
            nc.vector.tensor_scalar(tg0_t[:], ts_t[:], -1.0, 1.0, Alu.mult, Alu.add)

            pixi = persist.tile([P, cols], I32, tag="pixi")
            nc.vector.scalar_tensor_tensor(
                pixi[:], ys_t[:], float(W), xs_t[:], Alu.mult, Alu.add
            )

            # ---------------- charbonnier smoothness ----------------
            smooth_scratch = small.tile([P, 672], F32, tag="smooth_scratch")
            for ch in range(2):
                Fc = small.tile([P, 2 * W], F32, tag="Fc")
                nc.sync.dma_start(
                    out=Fc[:], in_=flow_r[ch].rearrange("(p r) w -> p (r w)", r=2)
                )
                Fs = small.tile([P - 1, 2 * W], F32, tag="Fs")
                nc.sync.dma_start(
                    out=Fs[:],
                    in_=flow_r[ch, 1 : 2 * P - 1, :].rearrange("(p r) w -> p (r w)", r=2),
                )
                dxa = small.tile([P, W], F32, tag="dxa")
                nc.vector.tensor_tensor(dxa[:], Fc[:, 0:W], Fc[:, W : 2 * W], Alu.subtract)
                nc.vector.scalar_tensor_tensor(
                    smooth_scratch[:, 0:W], dxa[:], 0.0, dxa[:], Alu.add, Alu.mult
                )
                nc.scalar.activation(
                    smooth_scratch[:, 0:W], smooth_scratch[:, 0:W], AF.Sqrt,
                    bias=c1em6[:], scale=1.0,
                    accum_out=out_t[:, 8 + 3 * ch : 9 + 3 * ch],
                )
                dxb = small.tile([P - 1, W], F32, tag="dxb")
                nc.vector.tensor_tensor(
                    dxb[:], Fc[: P - 1, W : 2 * W], Fs[:, W : 2 * W], Alu.subtract
                )
                nc.vector.scalar_tensor_tensor(
                    smooth_scratch[: P - 1, 0:W], dxb[:], 0.0, dxb[:], Alu.add, Alu.mult
                )
                nc.scalar.activation(
                    smooth_scratch[: P - 1, 0:W], smooth_scratch[: P - 1, 0:W], AF.Sqrt,
                    bias=c1em6[: P - 1], scale=1.0,
                    accum_out=out_t[: P - 1, 9 + 3 * ch : 10 + 3 * ch],
                )
                dy = small.tile([P, 2 * (W - 1)], F32, tag="dy_s")
                src_a = Fc[:].rearrange("p (r w) -> p r w", r=2)[:, :, 0 : W - 1]
                src_b = Fc[:].rearrange("p (r w) -> p r w", r=2)[:, :, 1:W]
                nc.vector.tensor_tensor(
                    dy[:].rearrange("p (r w) -> p r w", r=2), src_a, src_b, Alu.subtract
                )
                nc.vector.scalar_tensor_tensor(
                    smooth_scratch[:, 0 : 2 * (W - 1)], dy[:], 0.0, dy[:], Alu.add, Alu.mult
                )
                nc.scalar.activation(
                    smooth_scratch[:, 0 : 2 * (W - 1)], smooth_scratch[:, 0 : 2 * (W - 1)],
                    AF.Sqrt, bias=c1em6[:], scale=1.0,
                    accum_out=out_t[:, 10 + 3 * ch : 11 + 3 * ch],
                )

            # ---------------- SBUF image accumulators ----------------
            # acc[t][(v, h)] : [P, W] f32, positive-true images
            acc = {}
            for t in range(2):
                for v in VARIANTS:
                    for h in range(2):
                        acc[(t, v, h)] = accp.tile(
                            [P, W], F32, tag=f"acc_{t}_{v}_{h}", name=f"acc_{t}_{v}_{h}"
                        )

            # psum accumulators, shared between trefs (reused per block)
            ps = {}
            for v in VARIANTS:
                for h in range(2):
                    ps[(v, h)] = psum.tile([P, W], F32, tag=f"ps_{v}_{h}", name=f"ps_{v}_{h}")

            # ---------------- main block loop ----------------
            for blk in range(n_blocks):
                c0 = blk * blk_cols
                csl = slice(c0, c0 + blk_cols)

                # flow gather for this block: one indirect DMA per column
                fg = fgp.tile([P, 2 * blk_cols], F32, tag="fg")
                for j in range(blk_cols):
                    nc.gpsimd.indirect_dma_start(
                        out=fg[:, 2 * j : 2 * j + 2],
                        out_offset=None,
                        in_=flow_i[:],
                        in_offset=bass.IndirectOffsetOnAxis(
                            ap=pixi[:, c0 + j : c0 + j + 1], axis=0
                        ),
                    )
                fy = fg[:].rearrange("p (c two) -> p c two", two=2)[:, :, 0]
                fx = fg[:].rearrange("p (c two) -> p c two", two=2)[:, :, 1]

                # warped coords for both trefs (block slices)
                wyx = {}
                m1 = blkp.tile([P, blk_cols], F32, tag="m1")
                for t, tref in ((0, 1.0), (1, 0.0)):
                    wy = blkp.tile([P, blk_cols], F32, tag=f"wy{t}")
                    wx = blkp.tile([P, blk_cols], F32, tag=f"wx{t}")
                    nc.vector.scalar_tensor_tensor(
                        m1[:], ts_t[:, csl], float(tref), fy, Alu.subtract, Alu.mult
                    )
                    nc.vector.scalar_tensor_tensor(
                        wy[:], m1[:], -SCALE, ys_t[:, csl], Alu.mult, Alu.add
                    )
                    nc.vector.scalar_tensor_tensor(
                        m1[:], ts_t[:, csl], float(tref), fx, Alu.subtract, Alu.mult
                    )
                    nc.vector.scalar_tensor_tensor(
                        wx[:], m1[:], -SCALE, xs_t[:, csl], Alu.mult, Alu.add
                    )
                    wyx[t] = (wy, wx)

                for t in range(2):
                    wy, wx = wyx[t]
                    tg = ts_t if t == 0 else tg0_t
                    for p in range(blk_pairs):
                        cA = c0 + 2 * p
                        # pair tiles: two DoubleRow slots each
                        d_y = pairp.tile([P, 2 * H], F16, tag="d_y")
                        d_x = pairp.tile([P, 2 * W], F16, tag="d_x")
                        hy = pairp.tile([P, 2 * H], FP8, tag="hy")
                        hyt = pairp.tile([P, 2 * H], FP8, tag="hyt")
                        hx = pairp.tile([P, 2 * W], FP8, tag="hx")
                        hxp = pairp.tile([P, 2 * W], FP8, tag="hxp")
                        for j in range(2):
                            c = 2 * p + j
                            sy = slice(j * H, (j + 1) * H)
                            sx = slice(j * W, (j + 1) * W)
                            # d = k - w   (f16, DVE 4x)
                            nc.vector.tensor_scalar(
                                d_y[:, sy], iota_y[:], wy[:, c : c + 1], None, Alu.subtract
                            )
                            nc.vector.tensor_scalar(
                                d_x[:, sx], iota_x[:], wx[:, c : c + 1], None, Alu.subtract
                            )
                        # |d| via sign-bit clear (u16 view, DVE 4x, whole pair)
                        nc.vector.tensor_scalar(
                            d_y[:].bitcast(U16), d_y[:].bitcast(U16), 0x7FFF, None,
                            Alu.bitwise_and,
                        )
                        nc.vector.tensor_scalar(
                            d_x[:].bitcast(U16), d_x[:].bitcast(U16), 0x7FFF, None,
                            Alu.bitwise_and,
                        )
                        # hat' = min(a,1) - 1  (negative hats, fp8, whole pair)
                        nc.vector.tensor_scalar(
                            hy[:], d_y[:], 1.0, 1.0, Alu.min, Alu.subtract
                        )
                        nc.vector.tensor_scalar(
                            hx[:], d_x[:], 1.0, 1.0, Alu.min, Alu.subtract
                        )
                        # variant mults (per slot: per-event scalar)
                        for j in range(2):
                            c = 2 * p + j
                            cg = cA + j
                            sy = slice(j * H, (j + 1) * H)
                            sx = slice(j * W, (j + 1) * W)
                            if hyt_act:
                                nc.scalar.activation(
                                    hyt[:, sy], hy[:, sy], AF.Copy,
                                    bias=0.0, scale=tg[:, cg : cg + 1],
                                )
                            else:
                                nc.vector.tensor_scalar(
                                    hyt[:, sy], hy[:, sy], tg[:, cg : cg + 1], None,
                                    Alu.mult,
                                )
                            if hxp_act:
                                nc.scalar.activation(
                                    hxp[:, sx], hx[:, sx], AF.Copy,
                                    bias=0.0, scale=pm_t[:, cg : cg + 1],
                                )
                            else:
                                nc.vector.tensor_scalar(
                                    hxp[:, sx], hx[:, sx], pm_t[:, cg : cg + 1], None,
                                    Alu.mult,
                                )
                        # 8 DoubleRow matmuls: contract both slots (256 events)
                        start = p == 0
                        stop = p == blk_pairs - 1
                        hyr = hy[:].rearrange("p (two m) -> p two m", two=2)
                        hytr = hyt[:].rearrange("p (two m) -> p two m", two=2)
                        hxr = hx[:].rearrange("p (two n) -> p two n", two=2)
                        hxpr = hxp[:].rearrange("p (two n) -> p two n", two=2)
                        for h in range(2):
                            sh = slice(h * P, (h + 1) * P)
                            for v, lhs, rhs in (
                                ("all", hyr, hxr),
                                ("p0", hyr, hxpr),
                                ("tsall", hytr, hxr),
                                ("tsp0", hytr, hxpr),
                            ):
                                nc.tensor.matmul(
                                    ps[(v, h)][:],
                                    lhsT=lhs[:, :, sh],
                                    rhs=rhs,
                                    start=start,
                                    stop=stop,
                                    perf_mode=DR,
                                )
                    # flush psum -> SBUF accumulators (DVE)
                    for v in VARIANTS:
                        for h in range(2):
                            if blk == 0:
                                nc.vector.tensor_copy(
                                    acc[(t, v, h)][:], ps[(v, h)][:]
                                )
                            else:
                                nc.vector.tensor_tensor(
                                    acc[(t, v, h)][:], acc[(t, v, h)][:],
                                    ps[(v, h)][:], Alu.add,
                                )

            # ---------------- loss terms from accumulated images ----------------
            for t in range(2):
                for h in range(2):
                    s_all = acc[(t, "all", h)]
                    s_p0 = acc[(t, "p0", h)]
                    t_all = acc[(t, "tsall", h)]
                    t_p0 = acc[(t, "tsp0", h)]
                    # pol1 = all - p0 (in place into the "all" tiles)
                    nc.vector.tensor_tensor(s_all[:], s_all[:], s_p0[:], Alu.subtract)
                    nc.vector.tensor_tensor(t_all[:], t_all[:], t_p0[:], Alu.subtract)
                    for pi, (S_img, T_img) in enumerate(((s_p0, t_p0), (s_all, t_all))):
                        r = small.tile([P, W], F32, tag="recip")
                        nc.vector.tensor_scalar(r[:], S_img[:], 1e-9, None, Alu.add)
                        nc.vector.reciprocal(r[:], r[:])
                        q = small.tile([P, W], F32, tag="q")
                        nc.vector.tensor_tensor(q[:], T_img[:], r[:], Alu.mult)
                        col = 4 * t + 2 * h + pi
                        nc.vector.scalar_tensor_tensor(
                            r[:], q[:], 0.0, q[:], Alu.add, Alu.mult,
                            accum_out=out_t[:, col : col + 1],
                        )

            nc.sync.dma_start(out=out[:], in_=out_t[:])

    nc.finalize()
    return nc


# ---------------------------------------------------------------------------
# host-side wrapper
# ---------------------------------------------------------------------------

_CACHED = {}


def _get_nc():
    key = "full"
    if key not in _CACHED:
        _CACHED[key] = build_nc()
    return _CACHED[key]


def prep_core_inputs(flow_b, ev_b, pm_b):
    """Per-batch-element host prep: pure re-layout (sharding), no math beyond layout."""
    n = ev_b.shape[0]
    ev_t = np.empty((4, n), dtype=np.float32)
    ev_t[0] = ev_b[:, 0]
    ev_t[1] = ev_b[:, 1]
    ev_t[2] = ev_b[:, 2]
    ev_t[3] = pm_b[:, 0]
    flow_i = np.ascontiguousarray(
        np.stack([flow_b[1].reshape(-1), flow_b[0].reshape(-1)], axis=-1)
    ).astype(np.float32)
    flow_r = np.ascontiguousarray(flow_b).astype(np.float32)
    return {"ev_t": ev_t, "flow_i": flow_i, "flow_r": flow_r}


def finish(outs):
    """Combine per-core partials into the scalar loss."""
    total = np.float64(0.0)
    for o in outs:
        o = o.astype(np.float64)
        total += o[:, LOSS_COLS].sum() + REG_WEIGHT * o[:, SMOOTH_COLS].sum()
    return np.float32(total)


def kernel(flow, event_list, pol_mask):
    from concourse.bass_utils import run_bass_kernel_spmd

    flow = np.asarray(flow)
    event_list = np.asarray(event_list)
    pol_mask = np.asarray(pol_mask)
    nc = _get_nc()
    in_maps = [
        prep_core_inputs(flow[b], event_list[b], pol_mask[b]) for b in range(B)
    ]
    res = run_bass_kernel_spmd(nc, in_maps, list(range(B)))
    outs = [res.results[b]["out"] for b in range(B)]
    return finish(outs)


if __name__ == "__main__":
    rng = np.random.default_rng(0)
    flow = (0.05 * rng.standard_normal((B, 2, H, W))).astype(np.float32)
    ys = rng.integers(0, H, (B, N_FULL)).astype(np.float32)
    xs = rng.integers(0, W, (B, N_FULL)).astype(np.float32)
    ts = rng.random((B, N_FULL), dtype=np.float32)
    pol = rng.integers(0, 2, (B, N_FULL))
    ev = np.stack([ts, ys, xs, pol * 2.0 - 1.0], axis=-1).astype(np.float32)
    pm = np.stack([(pol == 1), (pol == 0)], axis=-1).astype(np.float32)
    print(kernel(flow, ev, pm))


# revision 5
# speedup vs baseline: 18.5749x; 17.7620x over previous
"""Trainium2 Bass kernel for nn_EventWarping (contrast-maximization event warping loss).

Strategy (data-parallel over batch, one batch element per NeuronCore):
  - flow gather at integer event pixels: indirect DMA (per-event-column 8B row
    gather on the gpsimd software-DGE path), pipelined per 128-column block so
    hat building overlaps descriptor generation.
  - the 4 scatter-add histograms (iwe / iwe_ts x pol0 / all) are computed as
    sums of rank-1 outer products of bilinear "hat" row/col vectors, using
    fp8e4 DoubleRow matmuls (2 event-chunks contracted per pass at 0.5
    cycles/row -> 4x the bf16 streaming rate):
        hat'(d) = min(|d|,1) - 1 = -relu(1 - |d|)   (negative form, one
        tensor_scalar pass from |d|; products of two negative hats are
        positive so no sign fixup is needed)
        |d| built from d = k - w via a uint16 bitwise-and (sign-bit clear),
        both passes run in the DVE 4x perf mode.
  - PSUM has only 8 banks = one tref's 8 half-images, so each 128-column
    block runs tref=1 then tref=0 through the same banks, flushing into
    SBUF f32 accumulators between trefs.
  - per-tref loss terms and charbonnier smoothness reduced on device to
    per-partition partial sums; host sums 8 cores' partials.
"""

import sys

import numpy as np

sys.path.insert(0, "/opt/trn_rl_repo")

import concourse.bass as bass
import concourse.bacc as bacc
import concourse.tile as tile
from concourse import mybir
from concourse.alu_op_type import AluOpType as Alu

F32 = mybir.dt.float32
F16 = mybir.dt.float16
U16 = mybir.dt.uint16
FP8 = mybir.dt.float8e4
I32 = mybir.dt.int32
AF = mybir.ActivationFunctionType
DR = mybir.MatmulPerfMode.DoubleRow

H, W = 256, 336
SCALE = 336.0
REG_WEIGHT = 0.001
B = 8
N_FULL = 262144
P = 128

LOSS_COLS = list(range(8))        # 2 trefs x 2 pols x 2 y-halves
SMOOTH_COLS = list(range(8, 14))  # 2 ch x {dxA, dxB, dy}

VARIANTS = ("all", "p0", "tsall", "tsp0")


def build_nc(n_events=N_FULL, blk_cols=128, hyt_act=True, hxp_act=True):
    """Build the SPMD Bass program for one core / one batch element."""
    cols = n_events // P          # events laid out as [128, cols]
    n_blocks = cols // blk_cols
    assert blk_cols % 2 == 0
    blk_pairs = blk_cols // 2

    nc = bacc.Bacc("TRN2", target_bir_lowering=False, debug=False, num_devices=8)

    ev_t = nc.declare_dram_parameter("ev_t", [4, n_events], F32, isOutput=False)
    flow_i = nc.declare_dram_parameter("flow_i", [H * W, 2], F32, isOutput=False)
    flow_r = nc.declare_dram_parameter("flow_r", [2, H, W], F32, isOutput=False)
    out = nc.declare_dram_parameter("out", [P, 16], F32, isOutput=True)

    with tile.TileContext(nc) as tc:
        with (
            tc.tile_pool(name="persist", bufs=1) as persist,
            tc.tile_pool(name="acc", bufs=1) as accp,
            tc.tile_pool(name="blk", bufs=2) as blkp,
            tc.tile_pool(name="fgp", bufs=3) as fgp,
            tc.tile_pool(name="pair", bufs=4) as pairp,
            tc.tile_pool(name="small", bufs=1) as small,
            tc.tile_pool(name="psum", bufs=1, space="PSUM") as psum,
        ):
            # ---------------- constants ----------------
            iota_i = small.tile([P, W], I32, tag="iota_i")
            nc.gpsimd.iota(iota_i[:], pattern=[[1, W]], base=0, channel_multiplier=0)
            iota_y = small.tile([P, H], F16, tag="iota_y")
            iota_x = small.tile([P, W], F16, tag="iota_x")
            nc.vector.tensor_copy(iota_y[:], iota_i[:, :H])
            nc.vector.tensor_copy(iota_x[:], iota_i[:])

            out_t = small.tile([P, 16], F32, tag="out_t")
            nc.vector.memset(out_t[:], 0.0)
            c1em6 = small.tile([P, 1], F32, tag="c1em6")
            nc.vector.memset(c1em6[:], 1e-6)

            # ---------------- load event scalars ----------------
            def load_row(r, tag):
                t = persist.tile([P, cols], F32, tag=tag)
                nc.sync.dma_start(out=t[:], in_=ev_t[r].rearrange("(p c) -> p c", p=P))
                return t

            ts_t = load_row(0, "ts")
            ys_t = load_row(1, "ys")
            xs_t = load_row(2, "xs")
            pm_t = load_row(3, "pm")

            # per-event scalars: tg (per tref) lives in ts_t / tg0_t
            tg0_t = persist.tile([P, cols], F32, tag="tg0")
            nc.vector.tensor_scalar(tg0_t[:], ts_t[:], -1.0, 1.0, Alu.mult, Alu.add)

            pixi = persist.tile([P, cols], I32, tag="pixi")
            nc.vector.scalar_tensor_tensor(
                pixi[:], ys_t[:], float(W), xs_t[:], Alu.mult, Alu.add
            )

            # ---------------- charbonnier smoothness ----------------
            smooth_scratch = small.tile([P, 672], F32, tag="smooth_scratch")
            for ch in range(2):
                Fc = small.tile([P, 2 * W], F32, tag="Fc")
                nc.sync.dma_start(
                    out=Fc[:], in_=flow_r[ch].rearrange("(p r) w -> p (r w)", r=2)
                )
                Fs = small.tile([P - 1, 2 * W], F32, tag="Fs")
                nc.sync.dma_start(
                    out=Fs[:],
                    in_=flow_r[ch, 1 : 2 * P - 1, :].rearrange("(p r) w -> p (r w)", r=2),
                )
                dxa = small.tile([P, W], F32, tag="dxa")
                nc.vector.tensor_tensor(dxa[:], Fc[:, 0:W], Fc[:, W : 2 * W], Alu.subtract)
                nc.vector.scalar_tensor_tensor(
                    smooth_scratch[:, 0:W], dxa[:], 0.0, dxa[:], Alu.add, Alu.mult
                )
                nc.scalar.activation(
                    smooth_scratch[:, 0:W], smooth_scratch[:, 0:W], AF.Sqrt,
                    bias=c1em6[:], scale=1.0,
                    accum_out=out_t[:, 8 + 3 * ch : 9 + 3 * ch],
                )
                dxb = small.tile([P - 1, W], F32, tag="dxb")
                nc.vector.tensor_tensor(
                    dxb[:], Fc[: P - 1, W : 2 * W], Fs[:, W : 2 * W], Alu.subtract
                )
                nc.vector.scalar_tensor_tensor(
                    smooth_scratch[: P - 1, 0:W], dxb[:], 0.0, dxb[:], Alu.add, Alu.mult
                )
                nc.scalar.activation(
                    smooth_scratch[: P - 1, 0:W], smooth_scratch[: P - 1, 0:W], AF.Sqrt,
                    bias=c1em6[: P - 1], scale=1.0,
                    accum_out=out_t[: P - 1, 9 + 3 * ch : 10 + 3 * ch],
                )
                dy = small.tile([P, 2 * (W - 1)], F32, tag="dy_s")
                src_a = Fc[:].rearrange("p (r w) -> p r w", r=2)[:, :, 0 : W - 1]
                src_b = Fc[:].rearrange("p (r w) -> p r w", r=2)[:, :, 1:W]
                nc.vector.tensor_tensor(
                    dy[:].rearrange("p (r w) -> p r w", r=2), src_a, src_b, Alu.subtract
                )
                nc.vector.scalar_tensor_tensor(
                    smooth_scratch[:, 0 : 2 * (W - 1)], dy[:], 0.0, dy[:], Alu.add, Alu.mult
                )
                nc.scalar.activation(
                    smooth_scratch[:, 0 : 2 * (W - 1)], smooth_scratch[:, 0 : 2 * (W - 1)],
                    AF.Sqrt, bias=c1em6[:], scale=1.0,
                    accum_out=out_t[:, 10 + 3 * ch : 11 + 3 * ch],
                )

            # ---------------- SBUF image accumulators ----------------
            # acc[t][(v, h)] : [P, W] f32, positive-true images
            acc = {}
            for t in range(2):
                for v in VARIANTS:
                    for h in range(2):
                        acc[(t, v, h)] = accp.tile(
                            [P, W], F32, tag=f"acc_{t}_{v}_{h}", name=f"acc_{t}_{v}_{h}"
                        )

            # psum accumulators, shared between trefs (reused per block)
            ps = {}
            for v in VARIANTS:
                for h in range(2):
                    ps[(v, h)] = psum.tile([P, W], F32, tag=f"ps_{v}_{h}", name=f"ps_{v}_{h}")

            # ---------------- main block loop ----------------
            for blk in range(n_blocks):
                c0 = blk * blk_cols
                csl = slice(c0, c0 + blk_cols)

                # flow gather for this block: one indirect DMA per column
                fg = fgp.tile([P, 2 * blk_cols], F32, tag="fg")
                for j in range(blk_cols):
                    nc.gpsimd.indirect_dma_start(
                        out=fg[:, 2 * j : 2 * j + 2],
                        out_offset=None,
                        in_=flow_i[:],
                        in_offset=bass.IndirectOffsetOnAxis(
                            ap=pixi[:, c0 + j : c0 + j + 1], axis=0
                        ),
                    )
                fy = fg[:].rearrange("p (c two) -> p c two", two=2)[:, :, 0]
                fx = fg[:].rearrange("p (c two) -> p c two", two=2)[:, :, 1]

                # warped coords for both trefs (block slices)
                wyx = {}
                m1 = blkp.tile([P, blk_cols], F32, tag="m1")
                for t, tref in ((0, 1.0), (1, 0.0)):
                    wy = blkp.tile([P, blk_cols], F32, tag=f"wy{t}")
                    wx = blkp.tile([P, blk_cols], F32, tag=f"wx{t}")
                    nc.vector.scalar_tensor_tensor(
                        m1[:], ts_t[:, csl], float(tref), fy, Alu.subtract, Alu.mult
                    )
                    nc.vector.scalar_tensor_tensor(
                        wy[:], m1[:], -SCALE, ys_t[:, csl], Alu.mult, Alu.add
                    )
                    nc.vector.scalar_tensor_tensor(
                        m1[:], ts_t[:, csl], float(tref), fx, Alu.subtract, Alu.mult
                    )
                    nc.vector.scalar_tensor_tensor(
                        wx[:], m1[:], -SCALE, xs_t[:, csl], Alu.mult, Alu.add
                    )
                    wyx[t] = (wy, wx)

                for t in range(2):
                    wy, wx = wyx[t]
                    tg = ts_t if t == 0 else tg0_t
                    for p in range(blk_pairs):
                        cA = c0 + 2 * p
                        # pair tiles: two DoubleRow slots each
                        d_y = pairp.tile([P, 2 * H], F16, tag="d_y")
                        d_x = pairp.tile([P, 2 * W], F16, tag="d_x")
                        hy = pairp.tile([P, 2 * H], FP8, tag="hy")
                        hyt = pairp.tile([P, 2 * H], FP8, tag="hyt")
                        hx = pairp.tile([P, 2 * W], FP8, tag="hx")
                        hxp = pairp.tile([P, 2 * W], FP8, tag="hxp")
                        for j in range(2):
                            c = 2 * p + j
                            sy = slice(j * H, (j + 1) * H)
                            sx = slice(j * W, (j + 1) * W)
                            # d = k - w   (f16, DVE 4x)
                            nc.vector.tensor_scalar(
                                d_y[:, sy], iota_y[:], wy[:, c : c + 1], None, Alu.subtract
                            )
                            nc.vector.tensor_scalar(
                                d_x[:, sx], iota_x[:], wx[:, c : c + 1], None, Alu.subtract
                            )
                        # |d| via sign-bit clear (u16 view, DVE 4x, whole pair)
                        nc.vector.tensor_scalar(
                            d_y[:].bitcast(U16), d_y[:].bitcast(U16), 0x7FFF, None,
                            Alu.bitwise_and,
                        )
                        nc.vector.tensor_scalar(
                            d_x[:].bitcast(U16), d_x[:].bitcast(U16), 0x7FFF, None,
                            Alu.bitwise_and,
                        )
                        # hat' = min(a,1) - 1  (negative hats, fp8, whole pair)
                        nc.vector.tensor_scalar(
                            hy[:], d_y[:], 1.0, 1.0, Alu.min, Alu.subtract
                        )
                        nc.vector.tensor_scalar(
                            hx[:], d_x[:], 1.0, 1.0, Alu.min, Alu.subtract
                        )
                        # variant mults (per slot: per-event scalar); spread
                        # across ACT (slot 0) and DVE (slot 1) for balance
                        for j in range(2):
                            c = 2 * p + j
                            cg = cA + j
                            sy = slice(j * H, (j + 1) * H)
                            sx = slice(j * W, (j + 1) * W)
                            if hyt_act:
                                nc.scalar.activation(
                                    hyt[:, sy], hy[:, sy], AF.Copy,
                                    bias=0.0, scale=tg[:, cg : cg + 1],
                                )
                            else:
                                nc.vector.tensor_scalar(
                                    hyt[:, sy], hy[:, sy], tg[:, cg : cg + 1], None,
                                    Alu.mult,
                                )
                            if hxp_act:
                                nc.scalar.activation(
                                    hxp[:, sx], hx[:, sx], AF.Copy,
                                    bias=0.0, scale=pm_t[:, cg : cg + 1],
                                )
                            else:
                                nc.vector.tensor_scalar(
                                    hxp[:, sx], hx[:, sx], pm_t[:, cg : cg + 1], None,
                                    Alu.mult,
                                )
                        # 8 DoubleRow matmuls: contract both slots (256 events)
                        start = p == 0
                        stop = p == blk_pairs - 1
                        hyr = hy[:].rearrange("p (two m) -> p two m", two=2)
                        hytr = hyt[:].rearrange("p (two m) -> p two m", two=2)
                        hxr = hx[:].rearrange("p (two n) -> p two n", two=2)
                        hxpr = hxp[:].rearrange("p (two n) -> p two n", two=2)
                        for h in range(2):
                            sh = slice(h * P, (h + 1) * P)
                            for v, lhs, rhs in (
                                ("all", hyr, hxr),
                                ("p0", hyr, hxpr),
                                ("tsall", hytr, hxr),
                                ("tsp0", hytr, hxpr),
                            ):
                                nc.tensor.matmul(
                                    ps[(v, h)][:],
                                    lhsT=lhs[:, :, sh],
                                    rhs=rhs,
                                    start=start,
                                    stop=stop,
                                    perf_mode=DR,
                                )
                    # flush psum -> SBUF accumulators (DVE)
                    for v in VARIANTS:
                        for h in range(2):
                            if blk == 0:
                                nc.vector.tensor_copy(
                                    acc[(t, v, h)][:], ps[(v, h)][:]
                                )
                            else:
                                nc.vector.tensor_tensor(
                                    acc[(t, v, h)][:], acc[(t, v, h)][:],
                                    ps[(v, h)][:], Alu.add,
                                )

            # ---------------- loss terms from accumulated images ----------------
            for t in range(2):
                for h in range(2):
                    s_all = acc[(t, "all", h)]
                    s_p0 = acc[(t, "p0", h)]
                    t_all = acc[(t, "tsall", h)]
                    t_p0 = acc[(t, "tsp0", h)]
                    # pol1 = all - p0 (in place into the "all" tiles)
                    nc.vector.tensor_tensor(s_all[:], s_all[:], s_p0[:], Alu.subtract)
                    nc.vector.tensor_tensor(t_all[:], t_all[:], t_p0[:], Alu.subtract)
                    for pi, (S_img, T_img) in enumerate(((s_p0, t_p0), (s_all, t_all))):
                        r = small.tile([P, W], F32, tag="recip")
                        nc.vector.tensor_scalar(r[:], S_img[:], 1e-9, None, Alu.add)
                        nc.vector.reciprocal(r[:], r[:])
                        q = small.tile([P, W], F32, tag="q")
                        nc.vector.tensor_tensor(q[:], T_img[:], r[:], Alu.mult)
                        col = 4 * t + 2 * h + pi
                        nc.vector.scalar_tensor_tensor(
                            r[:], q[:], 0.0, q[:], Alu.add, Alu.mult,
                            accum_out=out_t[:, col : col + 1],
                        )

            nc.sync.dma_start(out=out[:], in_=out_t[:])

    nc.finalize()
    return nc


# ---------------------------------------------------------------------------
# host-side wrapper
# ---------------------------------------------------------------------------

_CACHED = {}


def _get_nc():
    key = "full"
    if key not in _CACHED:
        _CACHED[key] = build_nc()
    return _CACHED[key]


def prep_core_inputs(flow_b, ev_b, pm_b):
    """Per-batch-element host prep: pure re-layout (sharding), no math beyond layout."""
    n = ev_b.shape[0]
    ev_t = np.empty((4, n), dtype=np.float32)
    ev_t[0] = ev_b[:, 0]
    ev_t[1] = ev_b[:, 1]
    ev_t[2] = ev_b[:, 2]
    ev_t[3] = pm_b[:, 0]
    flow_i = np.ascontiguousarray(
        np.stack([flow_b[1].reshape(-1), flow_b[0].reshape(-1)], axis=-1)
    ).astype(np.float32)
    flow_r = np.ascontiguousarray(flow_b).astype(np.float32)
    return {"ev_t": ev_t, "flow_i": flow_i, "flow_r": flow_r}


def finish(outs):
    """Combine per-core partials into the scalar loss."""
    total = np.float64(0.0)
    for o in outs:
        o = o.astype(np.float64)
        total += o[:, LOSS_COLS].sum() + REG_WEIGHT * o[:, SMOOTH_COLS].sum()
    return np.float32(total)


def kernel(flow, event_list, pol_mask):
    from concourse.bass_utils import run_bass_kernel_spmd

    flow = np.asarray(flow)
    event_list = np.asarray(event_list)
    pol_mask = np.asarray(pol_mask)
    nc = _get_nc()
    in_maps = [
        prep_core_inputs(flow[b], event_list[b], pol_mask[b]) for b in range(B)
    ]
    res = run_bass_kernel_spmd(nc, in_maps, list(range(B)))
    outs = [res.results[b]["out"] for b in range(B)]
    return finish(outs)


if __name__ == "__main__":
    rng = np.random.default_rng(0)
    flow = (0.05 * rng.standard_normal((B, 2, H, W))).astype(np.float32)
    ys = rng.integers(0, H, (B, N_FULL)).astype(np.float32)
    xs = rng.integers(0, W, (B, N_FULL)).astype(np.float32)
    ts = rng.random((B, N_FULL), dtype=np.float32)
    pol = rng.integers(0, 2, (B, N_FULL))
    ev = np.stack([ts, ys, xs, pol * 2.0 - 1.0], axis=-1).astype(np.float32)
    pm = np.stack([(pol == 1), (pol == 0)], axis=-1).astype(np.float32)
    print(kernel(flow, ev, pm))


# revision 13
# speedup vs baseline: 18.5858x; 1.0006x over previous
"""Trainium2 Bass kernel for nn_EventWarping (contrast-maximization event warping loss).

Strategy (data-parallel over batch, one batch element per NeuronCore):
  - flow gather at integer event pixels: indirect DMA (per-event-column 8B row
    gather on the gpsimd software-DGE path), pipelined per 128-column block so
    hat building overlaps descriptor generation.
  - the 4 scatter-add histograms (iwe / iwe_ts x pol0 / all) are computed as
    sums of rank-1 outer products of bilinear "hat" row/col vectors, using
    fp8e4 DoubleRow matmuls (2 event-chunks contracted per pass at 0.5
    cycles/row -> 4x the bf16 streaming rate):
        hat'(d) = min(|d|,1) - 1 = -relu(1 - |d|)   (negative form, one
        tensor_scalar pass from |d|; products of two negative hats are
        positive so no sign fixup is needed)
        |d| built from d = k - w via a uint16 bitwise-and (sign-bit clear),
        both passes run in the DVE 4x perf mode.
  - PSUM has only 8 banks = one tref's 8 half-images, so each 128-column
    block runs tref=1 then tref=0 through the same banks, flushing into
    SBUF f32 accumulators between trefs.
  - per-tref loss terms and charbonnier smoothness reduced on device to
    per-partition partial sums; host sums 8 cores' partials.
"""

import sys

import numpy as np

sys.path.insert(0, "/opt/trn_rl_repo")

import concourse.bass as bass
import concourse.bacc as bacc
import concourse.tile as tile
from concourse import mybir
from concourse.alu_op_type import AluOpType as Alu

F32 = mybir.dt.float32
F16 = mybir.dt.float16
U16 = mybir.dt.uint16
FP8 = mybir.dt.float8e4
I32 = mybir.dt.int32
AF = mybir.ActivationFunctionType
DR = mybir.MatmulPerfMode.DoubleRow

H, W = 256, 336
SCALE = 336.0
REG_WEIGHT = 0.001
B = 8
N_FULL = 262144
P = 128

LOSS_COLS = list(range(8))        # 2 trefs x 2 pols x 2 y-halves
SMOOTH_COLS = list(range(8, 14))  # 2 ch x {dxA, dxB, dy}

VARIANTS = ("all", "p0", "tsall", "tsp0")


def build_nc(n_events=N_FULL, blk_cols=64, hyt_act=True, hxp_act=True):
    """Build the SPMD Bass program for one core / one batch element."""
    cols = n_events // P          # events laid out as [128, cols]
    n_blocks = cols // blk_cols
    assert blk_cols % 2 == 0
    blk_pairs = blk_cols // 2

    nc = bacc.Bacc("TRN2", target_bir_lowering=False, debug=False, num_devices=8)

    ev_t = nc.declare_dram_parameter("ev_t", [4, n_events], F32, isOutput=False)
    flow_i = nc.declare_dram_parameter("flow_i", [H * W, 2], F32, isOutput=False)
    flow_r = nc.declare_dram_parameter("flow_r", [2, H, W], F32, isOutput=False)
    out = nc.declare_dram_parameter("out", [P, 16], F32, isOutput=True)

    with tile.TileContext(nc) as tc:
        with (
            tc.tile_pool(name="persist", bufs=1) as persist,
            tc.tile_pool(name="acc", bufs=1) as accp,
            tc.tile_pool(name="blk", bufs=2) as blkp,
            tc.tile_pool(name="fgp", bufs=3) as fgp,
            tc.tile_pool(name="pair", bufs=4) as pairp,
            tc.tile_pool(name="small", bufs=1) as small,
            tc.tile_pool(name="psum", bufs=1, space="PSUM") as psum,
        ):
            # ---------------- constants ----------------
            iota_i = small.tile([P, W], I32, tag="iota_i")
            nc.gpsimd.iota(iota_i[:], pattern=[[1, W]], base=0, channel_multiplier=0)
            iota_y = small.tile([P, H], F16, tag="iota_y")
            iota_x = small.tile([P, W], F16, tag="iota_x")
            nc.vector.tensor_copy(iota_y[:], iota_i[:, :H])
            nc.vector.tensor_copy(iota_x[:], iota_i[:])

            out_t = small.tile([P, 16], F32, tag="out_t")
            nc.vector.memset(out_t[:], 0.0)
            c1em6 = small.tile([P, 1], F32, tag="c1em6")
            nc.vector.memset(c1em6[:], 1e-6)

            # ---------------- load event scalars ----------------
            def load_row(r, tag):
                t = persist.tile([P, cols], F32, tag=tag)
                nc.sync.dma_start(out=t[:], in_=ev_t[r].rearrange("(p c) -> p c", p=P))
                return t

            ts_t = load_row(0, "ts")
            ys_t = load_row(1, "ys")
            xs_t = load_row(2, "xs")
            pm_t = load_row(3, "pm")

            # per-event scalars: tg (per tref) lives in ts_t / tg0_t
            tg0_t = persist.tile([P, cols], F32, tag="tg0")
            nc.vector.tensor_scalar(tg0_t[:], ts_t[:], -1.0, 1.0, Alu.mult, Alu.add)

            pixi = persist.tile([P, cols], I32, tag="pixi")
            nc.vector.scalar_tensor_tensor(
                pixi[:], ys_t[:], float(W), xs_t[:], Alu.mult, Alu.add
            )

            # ---------------- charbonnier smoothness ----------------
            smooth_scratch = small.tile([P, 672], F32, tag="smooth_scratch")
            for ch in range(2):
                Fc = small.tile([P, 2 * W], F32, tag="Fc")
                nc.sync.dma_start(
                    out=Fc[:], in_=flow_r[ch].rearrange("(p r) w -> p (r w)", r=2)
                )
                Fs = small.tile([P - 1, 2 * W], F32, tag="Fs")
                nc.sync.dma_start(
                    out=Fs[:],
                    in_=flow_r[ch, 1 : 2 * P - 1, :].rearrange("(p r) w -> p (r w)", r=2),
                )
                dxa = small.tile([P, W], F32, tag="dxa")
                nc.vector.tensor_tensor(dxa[:], Fc[:, 0:W], Fc[:, W : 2 * W], Alu.subtract)
                nc.vector.scalar_tensor_tensor(
                    smooth_scratch[:, 0:W], dxa[:], 0.0, dxa[:], Alu.add, Alu.mult
                )
                nc.scalar.activation(
                    smooth_scratch[:, 0:W], smooth_scratch[:, 0:W], AF.Sqrt,
                    bias=c1em6[:], scale=1.0,
                    accum_out=out_t[:, 8 + 3 * ch : 9 + 3 * ch],
                )
                dxb = small.tile([P - 1, W], F32, tag="dxb")
                nc.vector.tensor_tensor(
                    dxb[:], Fc[: P - 1, W : 2 * W], Fs[:, W : 2 * W], Alu.subtract
                )
                nc.vector.scalar_tensor_tensor(
                    smooth_scratch[: P - 1, 0:W], dxb[:], 0.0, dxb[:], Alu.add, Alu.mult
                )
                nc.scalar.activation(
                    smooth_scratch[: P - 1, 0:W], smooth_scratch[: P - 1, 0:W], AF.Sqrt,
                    bias=c1em6[: P - 1], scale=1.0,
                    accum_out=out_t[: P - 1, 9 + 3 * ch : 10 + 3 * ch],
                )
                dy = small.tile([P, 2 * (W - 1)], F32, tag="dy_s")
                src_a = Fc[:].rearrange("p (r w) -> p r w", r=2)[:, :, 0 : W - 1]
                src_b = Fc[:].rearrange("p (r w) -> p r w", r=2)[:, :, 1:W]
                nc.vector.tensor_tensor(
                    dy[:].rearrange("p (r w) -> p r w", r=2), src_a, src_b, Alu.subtract
                )
                nc.vector.scalar_tensor_tensor(
                    smooth_scratch[:, 0 : 2 * (W - 1)], dy[:], 0.0, dy[:], Alu.add, Alu.mult
                )
                nc.scalar.activation(
                    smooth_scratch[:, 0 : 2 * (W - 1)], smooth_scratch[:, 0 : 2 * (W - 1)],
                    AF.Sqrt, bias=c1em6[:], scale=1.0,
                    accum_out=out_t[:, 10 + 3 * ch : 11 + 3 * ch],
                )

            # ---------------- SBUF image accumulators ----------------
            # acc[t][(v, h)] : [P, W] f32, positive-true images
            acc = {}
            for t in range(2):
                for v in VARIANTS:
                    for h in range(2):
                        acc[(t, v, h)] = accp.tile(
                            [P, W], F32, tag=f"acc_{t}_{v}_{h}", name=f"acc_{t}_{v}_{h}"
                        )

            # psum accumulators, shared between trefs (reused per block)
            ps = {}
            for v in VARIANTS:
                for h in range(2):
                    ps[(v, h)] = psum.tile([P, W], F32, tag=f"ps_{v}_{h}", name=f"ps_{v}_{h}")

            # ---------------- main block loop ----------------
            for blk in range(n_blocks):
                c0 = blk * blk_cols
                csl = slice(c0, c0 + blk_cols)

                # flow gather for this block: one indirect DMA per column
                fg = fgp.tile([P, 2 * blk_cols], F32, tag="fg")
                for j in range(blk_cols):
                    nc.gpsimd.indirect_dma_start(
                        out=fg[:, 2 * j : 2 * j + 2],
                        out_offset=None,
                        in_=flow_i[:],
                        in_offset=bass.IndirectOffsetOnAxis(
                            ap=pixi[:, c0 + j : c0 + j + 1], axis=0
                        ),
                    )
                fy = fg[:].rearrange("p (c two) -> p c two", two=2)[:, :, 0]
                fx = fg[:].rearrange("p (c two) -> p c two", two=2)[:, :, 1]

                # warped coords for both trefs (block slices); scalar operands
                # must stay f32 (ISA: PTR scalars are read as float32)
                wyx = {}
                m1 = blkp.tile([P, blk_cols], F32, tag="m1")
                for t, tref in ((0, 1.0), (1, 0.0)):
                    wy = blkp.tile([P, blk_cols], F32, tag=f"wy{t}")
                    wx = blkp.tile([P, blk_cols], F32, tag=f"wx{t}")
                    nc.vector.scalar_tensor_tensor(
                        m1[:], ts_t[:, csl], float(tref), fy, Alu.subtract, Alu.mult
                    )
                    nc.vector.scalar_tensor_tensor(
                        wy[:], m1[:], -SCALE, ys_t[:, csl], Alu.mult, Alu.add
                    )
                    nc.vector.scalar_tensor_tensor(
                        m1[:], ts_t[:, csl], float(tref), fx, Alu.subtract, Alu.mult
                    )
                    nc.vector.scalar_tensor_tensor(
                        wx[:], m1[:], -SCALE, xs_t[:, csl], Alu.mult, Alu.add
                    )
                    wyx[t] = (wy, wx)

                for t in range(2):
                    wy, wx = wyx[t]
                    tg = ts_t if t == 0 else tg0_t
                    for p in range(blk_pairs):
                        cA = c0 + 2 * p
                        # pair tiles: two DoubleRow slots each
                        d_y = pairp.tile([P, 2 * H], F16, tag="d_y")
                        d_x = pairp.tile([P, 2 * W], F16, tag="d_x")
                        hy = pairp.tile([P, 2 * H], FP8, tag="hy")
                        hyt = pairp.tile([P, 2 * H], FP8, tag="hyt")
                        hx = pairp.tile([P, 2 * W], FP8, tag="hx")
                        hxp = pairp.tile([P, 2 * W], FP8, tag="hxp")
                        for j in range(2):
                            c = 2 * p + j
                            sy = slice(j * H, (j + 1) * H)
                            sx = slice(j * W, (j + 1) * W)
                            # d = k - w   (f16)
                            nc.vector.tensor_scalar(
                                d_y[:, sy], iota_y[:], wy[:, c : c + 1], None, Alu.subtract
                            )
                            nc.vector.tensor_scalar(
                                d_x[:, sx], iota_x[:], wx[:, c : c + 1], None, Alu.subtract
                            )
                        # |d| via sign-bit clear (u16 view, whole pair)
                        nc.vector.tensor_scalar(
                            d_y[:].bitcast(U16), d_y[:].bitcast(U16), 0x7FFF, None,
                            Alu.bitwise_and,
                        )
                        nc.vector.tensor_scalar(
                            d_x[:].bitcast(U16), d_x[:].bitcast(U16), 0x7FFF, None,
                            Alu.bitwise_and,
                        )
                        # hat' = min(a,1) - 1  (negative hats, fp8, whole pair)
                        nc.vector.tensor_scalar(
                            hy[:], d_y[:], 1.0, 1.0, Alu.min, Alu.subtract
                        )
                        nc.vector.tensor_scalar(
                            hx[:], d_x[:], 1.0, 1.0, Alu.min, Alu.subtract
                        )
                        # variant mults (per slot: per-event scalar) on ACT
                        for j in range(2):
                            cg = cA + j
                            sy = slice(j * H, (j + 1) * H)
                            sx = slice(j * W, (j + 1) * W)
                            if hyt_act:
                                nc.scalar.activation(
                                    hyt[:, sy], hy[:, sy], AF.Copy,
                                    bias=0.0, scale=tg[:, cg : cg + 1],
                                )
                            else:
                                nc.vector.tensor_scalar(
                                    hyt[:, sy], hy[:, sy], tg[:, cg : cg + 1], None,
                                    Alu.mult,
                                )
                            if hxp_act:
                                nc.scalar.activation(
                                    hxp[:, sx], hx[:, sx], AF.Copy,
                                    bias=0.0, scale=pm_t[:, cg : cg + 1],
                                )
                            else:
                                nc.vector.tensor_scalar(
                                    hxp[:, sx], hx[:, sx], pm_t[:, cg : cg + 1], None,
                                    Alu.mult,
                                )
                        # 8 DoubleRow matmuls: contract both slots (256 events)
                        start = p == 0
                        stop = p == blk_pairs - 1
                        hyr = hy[:].rearrange("p (two m) -> p two m", two=2)
                        hytr = hyt[:].rearrange("p (two m) -> p two m", two=2)
                        hxr = hx[:].rearrange("p (two n) -> p two n", two=2)
                        hxpr = hxp[:].rearrange("p (two n) -> p two n", two=2)
                        for h in range(2):
                            sh = slice(h * P, (h + 1) * P)
                            for v, lhs, rhs in (
                                ("all", hyr, hxr),
                                ("p0", hyr, hxpr),
                                ("tsall", hytr, hxr),
                                ("tsp0", hytr, hxpr),
                            ):
                                nc.tensor.matmul(
                                    ps[(v, h)][:],
                                    lhsT=lhs[:, :, sh],
                                    rhs=rhs,
                                    start=start,
                                    stop=stop,
                                    perf_mode=DR,
                                )
                    # flush psum -> SBUF accumulators (DVE)
                    for v in VARIANTS:
                        for h in range(2):
                            if blk == 0:
                                nc.vector.tensor_copy(
                                    acc[(t, v, h)][:], ps[(v, h)][:]
                                )
                            else:
                                nc.vector.tensor_tensor(
                                    acc[(t, v, h)][:], acc[(t, v, h)][:],
                                    ps[(v, h)][:], Alu.add,
                                )

            # ---------------- loss terms from accumulated images ----------------
            for t in range(2):
                for h in range(2):
                    s_all = acc[(t, "all", h)]
                    s_p0 = acc[(t, "p0", h)]
                    t_all = acc[(t, "tsall", h)]
                    t_p0 = acc[(t, "tsp0", h)]
                    # pol1 = all - p0 (in place into the "all" tiles)
                    nc.vector.tensor_tensor(s_all[:], s_all[:], s_p0[:], Alu.subtract)
                    nc.vector.tensor_tensor(t_all[:], t_all[:], t_p0[:], Alu.subtract)
                    for pi, (S_img, T_img) in enumerate(((s_p0, t_p0), (s_all, t_all))):
                        r = small.tile([P, W], F32, tag="recip")
                        nc.vector.tensor_scalar(r[:], S_img[:], 1e-9, None, Alu.add)
                        nc.vector.reciprocal(r[:], r[:])
                        q = small.tile([P, W], F32, tag="q")
                        nc.vector.tensor_tensor(q[:], T_img[:], r[:], Alu.mult)
                        col = 4 * t + 2 * h + pi
                        nc.vector.scalar_tensor_tensor(
                            r[:], q[:], 0.0, q[:], Alu.add, Alu.mult,
                            accum_out=out_t[:, col : col + 1],
                        )

            nc.sync.dma_start(out=out[:], in_=out_t[:])

    nc.finalize()
    return nc


# ---------------------------------------------------------------------------
# host-side wrapper
# ---------------------------------------------------------------------------

_CACHED = {}


def _get_nc():
    key = "full"
    if key not in _CACHED:
        _CACHED[key] = build_nc()
    return _CACHED[key]


def prep_core_inputs(flow_b, ev_b, pm_b):
    """Per-batch-element host prep: pure re-layout (sharding), no math beyond layout."""
    n = ev_b.shape[0]
    ev_t = np.empty((4, n), dtype=np.float32)
    ev_t[0] = ev_b[:, 0]
    ev_t[1] = ev_b[:, 1]
    ev_t[2] = ev_b[:, 2]
    ev_t[3] = pm_b[:, 0]
    flow_i = np.ascontiguousarray(
        np.stack([flow_b[1].reshape(-1), flow_b[0].reshape(-1)], axis=-1)
    ).astype(np.float32)
    flow_r = np.ascontiguousarray(flow_b).astype(np.float32)
    return {"ev_t": ev_t, "flow_i": flow_i, "flow_r": flow_r}


def finish(outs):
    """Combine per-core partials into the scalar loss."""
    total = np.float64(0.0)
    for o in outs:
        o = o.astype(np.float64)
        total += o[:, LOSS_COLS].sum() + REG_WEIGHT * o[:, SMOOTH_COLS].sum()
    return np.float32(total)


def kernel(flow, event_list, pol_mask):
    from concourse.bass_utils import run_bass_kernel_spmd

    flow = np.asarray(flow)
    event_list = np.asarray(event_list)
    pol_mask = np.asarray(pol_mask)
    nc = _get_nc()
    in_maps = [
        prep_core_inputs(flow[b], event_list[b], pol_mask[b]) for b in range(B)
    ]
    res = run_bass_kernel_spmd(nc, in_maps, list(range(B)))
    outs = [res.results[b]["out"] for b in range(B)]
    return finish(outs)


if __name__ == "__main__":
    rng = np.random.default_rng(0)
    flow = (0.05 * rng.standard_normal((B, 2, H, W))).astype(np.float32)
    ys = rng.integers(0, H, (B, N_FULL)).astype(np.float32)
    xs = rng.integers(0, W, (B, N_FULL)).astype(np.float32)
    ts = rng.random((B, N_FULL), dtype=np.float32)
    pol = rng.integers(0, 2, (B, N_FULL))
    ev = np.stack([ts, ys, xs, pol * 2.0 - 1.0], axis=-1).astype(np.float32)
    pm = np.stack([(pol == 1), (pol == 0)], axis=-1).astype(np.float32)
    print(kernel(flow, ev, pm))


# revision 14
# speedup vs baseline: 18.6624x; 1.0041x over previous
"""Trainium2 Bass kernel for nn_EventWarping (contrast-maximization event warping loss).

Strategy (data-parallel over batch, one batch element per NeuronCore):
  - flow gather at integer event pixels: indirect DMA (per-event-column 8B row
    gather on the gpsimd software-DGE path), pipelined per 128-column block so
    hat building overlaps descriptor generation.
  - the 4 scatter-add histograms (iwe / iwe_ts x pol0 / all) are computed as
    sums of rank-1 outer products of bilinear "hat" row/col vectors, using
    fp8e4 DoubleRow matmuls (2 event-chunks contracted per pass at 0.5
    cycles/row -> 4x the bf16 streaming rate):
        hat'(d) = min(|d|,1) - 1 = -relu(1 - |d|)   (negative form, one
        tensor_scalar pass from |d|; products of two negative hats are
        positive so no sign fixup is needed)
        |d| built from d = k - w via a uint16 bitwise-and (sign-bit clear),
        both passes run in the DVE 4x perf mode.
  - PSUM has only 8 banks = one tref's 8 half-images, so each 128-column
    block runs tref=1 then tref=0 through the same banks, flushing into
    SBUF f32 accumulators between trefs.
  - per-tref loss terms and charbonnier smoothness reduced on device to
    per-partition partial sums; host sums 8 cores' partials.
"""

import sys

import numpy as np

sys.path.insert(0, "/opt/trn_rl_repo")

import concourse.bass as bass
import concourse.bacc as bacc
import concourse.tile as tile
from concourse import mybir
from concourse.alu_op_type import AluOpType as Alu

F32 = mybir.dt.float32
F16 = mybir.dt.float16
U16 = mybir.dt.uint16
FP8 = mybir.dt.float8e4
I32 = mybir.dt.int32
AF = mybir.ActivationFunctionType
DR = mybir.MatmulPerfMode.DoubleRow

H, W = 256, 336
SCALE = 336.0
REG_WEIGHT = 0.001
B = 8
N_FULL = 262144
P = 128

LOSS_COLS = list(range(8))        # 2 trefs x 2 pols x 2 y-halves
SMOOTH_COLS = list(range(8, 14))  # 2 ch x {dxA, dxB, dy}

VARIANTS = ("all", "p0", "tsall", "tsp0")


def build_nc(n_events=N_FULL, blk_cols=64, hyt_act=True, hxp_act=True):
    """Build the SPMD Bass program for one core / one batch element."""
    cols = n_events // P          # events laid out as [128, cols]
    n_blocks = cols // blk_cols
    assert blk_cols % 2 == 0
    blk_pairs = blk_cols // 2

    nc = bacc.Bacc("TRN2", target_bir_lowering=False, debug=False, num_devices=8)

    ev_t = nc.declare_dram_parameter("ev_t", [4, n_events], F32, isOutput=False)
    flow_i = nc.declare_dram_parameter("flow_i", [H * W, 2], F32, isOutput=False)
    flow_r = nc.declare_dram_parameter("flow_r", [2, H, W], F32, isOutput=False)
    out = nc.declare_dram_parameter("out", [P, 16], F32, isOutput=True)

    with tile.TileContext(nc) as tc:
        with (
            tc.tile_pool(name="persist", bufs=1) as persist,
            tc.tile_pool(name="acc", bufs=1) as accp,
            tc.tile_pool(name="blk", bufs=3) as blkp,
            tc.tile_pool(name="fgp", bufs=4) as fgp,
            tc.tile_pool(name="pair", bufs=8) as pairp,
            tc.tile_pool(name="small", bufs=1) as small,
            tc.tile_pool(name="psum", bufs=1, space="PSUM") as psum,
        ):
            # ---------------- constants ----------------
            iota_i = small.tile([P, W], I32, tag="iota_i")
            nc.gpsimd.iota(iota_i[:], pattern=[[1, W]], base=0, channel_multiplier=0)
            iota_y = small.tile([P, H], F16, tag="iota_y")
            iota_x = small.tile([P, W], F16, tag="iota_x")
            nc.vector.tensor_copy(iota_y[:], iota_i[:, :H])
            nc.vector.tensor_copy(iota_x[:], iota_i[:])

            out_t = small.tile([P, 16], F32, tag="out_t")
            nc.vector.memset(out_t[:], 0.0)
            c1em6 = small.tile([P, 1], F32, tag="c1em6")
            nc.vector.memset(c1em6[:], 1e-6)

            # ---------------- load event scalars ----------------
            def load_row(r, tag):
                t = persist.tile([P, cols], F32, tag=tag)
                nc.sync.dma_start(out=t[:], in_=ev_t[r].rearrange("(p c) -> p c", p=P))
                return t

            ts_t = load_row(0, "ts")
            ys_t = load_row(1, "ys")
            xs_t = load_row(2, "xs")
            pm_t = load_row(3, "pm")

            # per-event scalars: tg (per tref) lives in ts_t / tg0_t
            tg0_t = persist.tile([P, cols], F32, tag="tg0")
            nc.vector.tensor_scalar(tg0_t[:], ts_t[:], -1.0, 1.0, Alu.mult, Alu.add)

            pixi = persist.tile([P, cols], I32, tag="pixi")
            nc.vector.scalar_tensor_tensor(
                pixi[:], ys_t[:], float(W), xs_t[:], Alu.mult, Alu.add
            )

            # ---------------- charbonnier smoothness ----------------
            smooth_scratch = small.tile([P, 672], F32, tag="smooth_scratch")
            for ch in range(2):
                Fc = small.tile([P, 2 * W], F32, tag="Fc")
                nc.sync.dma_start(
                    out=Fc[:], in_=flow_r[ch].rearrange("(p r) w -> p (r w)", r=2)
                )
                Fs = small.tile([P - 1, 2 * W], F32, tag="Fs")
                nc.sync.dma_start(
                    out=Fs[:],
                    in_=flow_r[ch, 1 : 2 * P - 1, :].rearrange("(p r) w -> p (r w)", r=2),
                )
                dxa = small.tile([P, W], F32, tag="dxa")
                nc.vector.tensor_tensor(dxa[:], Fc[:, 0:W], Fc[:, W : 2 * W], Alu.subtract)
                nc.vector.scalar_tensor_tensor(
                    smooth_scratch[:, 0:W], dxa[:], 0.0, dxa[:], Alu.add, Alu.mult
                )
                nc.scalar.activation(
                    smooth_scratch[:, 0:W], smooth_scratch[:, 0:W], AF.Sqrt,
                    bias=c1em6[:], scale=1.0,
                    accum_out=out_t[:, 8 + 3 * ch : 9 + 3 * ch],
                )
                dxb = small.tile([P - 1, W], F32, tag="dxb")
                nc.vector.tensor_tensor(
                    dxb[:], Fc[: P - 1, W : 2 * W], Fs[:, W : 2 * W], Alu.subtract
                )
                nc.vector.scalar_tensor_tensor(
                    smooth_scratch[: P - 1, 0:W], dxb[:], 0.0, dxb[:], Alu.add, Alu.mult
                )
                nc.scalar.activation(
                    smooth_scratch[: P - 1, 0:W], smooth_scratch[: P - 1, 0:W], AF.Sqrt,
                    bias=c1em6[: P - 1], scale=1.0,
                    accum_out=out_t[: P - 1, 9 + 3 * ch : 10 + 3 * ch],
                )
                dy = small.tile([P, 2 * (W - 1)], F32, tag="dy_s")
                src_a = Fc[:].rearrange("p (r w) -> p r w", r=2)[:, :, 0 : W - 1]
                src_b = Fc[:].rearrange("p (r w) -> p r w", r=2)[:, :, 1:W]
                nc.vector.tensor_tensor(
                    dy[:].rearrange("p (r w) -> p r w", r=2), src_a, src_b, Alu.subtract
                )
                nc.vector.scalar_tensor_tensor(
                    smooth_scratch[:, 0 : 2 * (W - 1)], dy[:], 0.0, dy[:], Alu.add, Alu.mult
                )
                nc.scalar.activation(
                    smooth_scratch[:, 0 : 2 * (W - 1)], smooth_scratch[:, 0 : 2 * (W - 1)],
                    AF.Sqrt, bias=c1em6[:], scale=1.0,
                    accum_out=out_t[:, 10 + 3 * ch : 11 + 3 * ch],
                )

            # ---------------- SBUF image accumulators ----------------
            # acc[t][(v, h)] : [P, W] f32, positive-true images
            acc = {}
            for t in range(2):
                for v in VARIANTS:
                    for h in range(2):
                        acc[(t, v, h)] = accp.tile(
                            [P, W], F32, tag=f"acc_{t}_{v}_{h}", name=f"acc_{t}_{v}_{h}"
                        )

            # psum accumulators, shared between trefs (reused per block)
            ps = {}
            for v in VARIANTS:
                for h in range(2):
                    ps[(v, h)] = psum.tile([P, W], F32, tag=f"ps_{v}_{h}", name=f"ps_{v}_{h}")

            # ---------------- main block loop ----------------
            for blk in range(n_blocks):
                c0 = blk * blk_cols
                csl = slice(c0, c0 + blk_cols)

                # flow gather for this block: one indirect DMA per column
                fg = fgp.tile([P, 2 * blk_cols], F32, tag="fg")
                for j in range(blk_cols):
                    nc.gpsimd.indirect_dma_start(
                        out=fg[:, 2 * j : 2 * j + 2],
                        out_offset=None,
                        in_=flow_i[:],
                        in_offset=bass.IndirectOffsetOnAxis(
                            ap=pixi[:, c0 + j : c0 + j + 1], axis=0
                        ),
                    )
                fy = fg[:].rearrange("p (c two) -> p c two", two=2)[:, :, 0]
                fx = fg[:].rearrange("p (c two) -> p c two", two=2)[:, :, 1]

                # warped coords for both trefs (block slices); scalar operands
                # must stay f32 (ISA: PTR scalars are read as float32)
                wyx = {}
                m1 = blkp.tile([P, blk_cols], F32, tag="m1")
                for t, tref in ((0, 1.0), (1, 0.0)):
                    wy = blkp.tile([P, blk_cols], F32, tag=f"wy{t}")
                    wx = blkp.tile([P, blk_cols], F32, tag=f"wx{t}")
                    nc.vector.scalar_tensor_tensor(
                        m1[:], ts_t[:, csl], float(tref), fy, Alu.subtract, Alu.mult
                    )
                    nc.vector.scalar_tensor_tensor(
                        wy[:], m1[:], -SCALE, ys_t[:, csl], Alu.mult, Alu.add
                    )
                    nc.vector.scalar_tensor_tensor(
                        m1[:], ts_t[:, csl], float(tref), fx, Alu.subtract, Alu.mult
                    )
                    nc.vector.scalar_tensor_tensor(
                        wx[:], m1[:], -SCALE, xs_t[:, csl], Alu.mult, Alu.add
                    )
                    wyx[t] = (wy, wx)

                for t in range(2):
                    wy, wx = wyx[t]
                    tg = ts_t if t == 0 else tg0_t
                    for p in range(blk_pairs):
                        cA = c0 + 2 * p
                        # pair tiles: two DoubleRow slots each
                        d_y = pairp.tile([P, 2 * H], F16, tag="d_y")
                        d_x = pairp.tile([P, 2 * W], F16, tag="d_x")
                        hy = pairp.tile([P, 2 * H], FP8, tag="hy")
                        hyt = pairp.tile([P, 2 * H], FP8, tag="hyt")
                        hx = pairp.tile([P, 2 * W], FP8, tag="hx")
                        hxp = pairp.tile([P, 2 * W], FP8, tag="hxp")
                        for j in range(2):
                            c = 2 * p + j
                            sy = slice(j * H, (j + 1) * H)
                            sx = slice(j * W, (j + 1) * W)
                            # d = k - w   (f16)
                            nc.vector.tensor_scalar(
                                d_y[:, sy], iota_y[:], wy[:, c : c + 1], None, Alu.subtract
                            )
                            nc.vector.tensor_scalar(
                                d_x[:, sx], iota_x[:], wx[:, c : c + 1], None, Alu.subtract
                            )
                        # |d| via sign-bit clear (u16 view, whole pair)
                        nc.vector.tensor_scalar(
                            d_y[:].bitcast(U16), d_y[:].bitcast(U16), 0x7FFF, None,
                            Alu.bitwise_and,
                        )
                        nc.vector.tensor_scalar(
                            d_x[:].bitcast(U16), d_x[:].bitcast(U16), 0x7FFF, None,
                            Alu.bitwise_and,
                        )
                        # hat' = min(a,1) - 1  (negative hats, fp8, whole pair)
                        nc.vector.tensor_scalar(
                            hy[:], d_y[:], 1.0, 1.0, Alu.min, Alu.subtract
                        )
                        nc.vector.tensor_scalar(
                            hx[:], d_x[:], 1.0, 1.0, Alu.min, Alu.subtract
                        )
                        # variant mults (per slot: per-event scalar) on ACT
                        for j in range(2):
                            cg = cA + j
                            sy = slice(j * H, (j + 1) * H)
                            sx = slice(j * W, (j + 1) * W)
                            if hyt_act:
                                nc.scalar.activation(
                                    hyt[:, sy], hy[:, sy], AF.Copy,
                                    bias=0.0, scale=tg[:, cg : cg + 1],
                                )
                            else:
                                nc.vector.tensor_scalar(
                                    hyt[:, sy], hy[:, sy], tg[:, cg : cg + 1], None,
                                    Alu.mult,
                                )
                            if hxp_act:
                                nc.scalar.activation(
                                    hxp[:, sx], hx[:, sx], AF.Copy,
                                    bias=0.0, scale=pm_t[:, cg : cg + 1],
                                )
                            else:
                                nc.vector.tensor_scalar(
                                    hxp[:, sx], hx[:, sx], pm_t[:, cg : cg + 1], None,
                                    Alu.mult,
                                )
                        # 8 DoubleRow matmuls: contract both slots (256 events)
                        start = p == 0
                        stop = p == blk_pairs - 1
                        hyr = hy[:].rearrange("p (two m) -> p two m", two=2)
                        hytr = hyt[:].rearrange("p (two m) -> p two m", two=2)
                        hxr = hx[:].rearrange("p (two n) -> p two n", two=2)
                        hxpr = hxp[:].rearrange("p (two n) -> p two n", two=2)
                        for h in range(2):
                            sh = slice(h * P, (h + 1) * P)
                            for v, lhs, rhs in (
                                ("all", hyr, hxr),
                                ("p0", hyr, hxpr),
                                ("tsall", hytr, hxr),
                                ("tsp0", hytr, hxpr),
                            ):
                                nc.tensor.matmul(
                                    ps[(v, h)][:],
                                    lhsT=lhs[:, :, sh],
                                    rhs=rhs,
                                    start=start,
                                    stop=stop,
                                    perf_mode=DR,
                                )
                    # flush psum -> SBUF accumulators (DVE)
                    for v in VARIANTS:
                        for h in range(2):
                            if blk == 0:
                                nc.vector.tensor_copy(
                                    acc[(t, v, h)][:], ps[(v, h)][:]
                                )
                            else:
                                nc.vector.tensor_tensor(
                                    acc[(t, v, h)][:], acc[(t, v, h)][:],
                                    ps[(v, h)][:], Alu.add,
                                )

            # ---------------- loss terms from accumulated images ----------------
            for t in range(2):
                for h in range(2):
                    s_all = acc[(t, "all", h)]
                    s_p0 = acc[(t, "p0", h)]
                    t_all = acc[(t, "tsall", h)]
                    t_p0 = acc[(t, "tsp0", h)]
                    # pol1 = all - p0 (in place into the "all" tiles)
                    nc.vector.tensor_tensor(s_all[:], s_all[:], s_p0[:], Alu.subtract)
                    nc.vector.tensor_tensor(t_all[:], t_all[:], t_p0[:], Alu.subtract)
                    for pi, (S_img, T_img) in enumerate(((s_p0, t_p0), (s_all, t_all))):
                        r = small.tile([P, W], F32, tag="recip")
                        nc.vector.tensor_scalar(r[:], S_img[:], 1e-9, None, Alu.add)
                        nc.vector.reciprocal(r[:], r[:])
                        q = small.tile([P, W], F32, tag="q")
                        nc.vector.tensor_tensor(q[:], T_img[:], r[:], Alu.mult)
                        col = 4 * t + 2 * h + pi
                        nc.vector.scalar_tensor_tensor(
                            r[:], q[:], 0.0, q[:], Alu.add, Alu.mult,
                            accum_out=out_t[:, col : col + 1],
                        )

            nc.sync.dma_start(out=out[:], in_=out_t[:])

    nc.finalize()
    return nc


# ---------------------------------------------------------------------------
# host-side wrapper
# ---------------------------------------------------------------------------

_CACHED = {}


def _get_nc():
    key = "full"
    if key not in _CACHED:
        _CACHED[key] = build_nc()
    return _CACHED[key]


def prep_core_inputs(flow_b, ev_b, pm_b):
    """Per-batch-element host prep: pure re-layout (sharding), no math beyond layout."""
    n = ev_b.shape[0]
    ev_t = np.empty((4, n), dtype=np.float32)
    ev_t[0] = ev_b[:, 0]
    ev_t[1] = ev_b[:, 1]
    ev_t[2] = ev_b[:, 2]
    ev_t[3] = pm_b[:, 0]
    flow_i = np.ascontiguousarray(
        np.stack([flow_b[1].reshape(-1), flow_b[0].reshape(-1)], axis=-1)
    ).astype(np.float32)
    flow_r = np.ascontiguousarray(flow_b).astype(np.float32)
    return {"ev_t": ev_t, "flow_i": flow_i, "flow_r": flow_r}


def finish(outs):
    """Combine per-core partials into the scalar loss."""
    total = np.float64(0.0)
    for o in outs:
        o = o.astype(np.float64)
        total += o[:, LOSS_COLS].sum() + REG_WEIGHT * o[:, SMOOTH_COLS].sum()
    return np.float32(total)


def kernel(flow, event_list, pol_mask):
    from concourse.bass_utils import run_bass_kernel_spmd

    flow = np.asarray(flow)
    event_list = np.asarray(event_list)
    pol_mask = np.asarray(pol_mask)
    nc = _get_nc()
    in_maps = [
        prep_core_inputs(flow[b], event_list[b], pol_mask[b]) for b in range(B)
    ]
    res = run_bass_kernel_spmd(nc, in_maps, list(range(B)))
    outs = [res.results[b]["out"] for b in range(B)]
    return finish(outs)


if __name__ == "__main__":
    rng = np.random.default_rng(0)
    flow = (0.05 * rng.standard_normal((B, 2, H, W))).astype(np.float32)
    ys = rng.integers(0, H, (B, N_FULL)).astype(np.float32)
    xs = rng.integers(0, W, (B, N_FULL)).astype(np.float32)
    ts = rng.random((B, N_FULL), dtype=np.float32)
    pol = rng.integers(0, 2, (B, N_FULL))
    ev = np.stack([ts, ys, xs, pol * 2.0 - 1.0], axis=-1).astype(np.float32)
    pm = np.stack([(pol == 1), (pol == 0)], axis=-1).astype(np.float32)
    print(kernel(flow, ev, pm))


# revision 20
# speedup vs baseline: 18.7544x; 1.0049x over previous
"""Trainium2 Bass kernel for nn_EventWarping (contrast-maximization event warping loss).

Strategy (data-parallel over batch, one batch element per NeuronCore):
  - flow gather at integer event pixels: indirect DMA (per-event-column 8B row
    gather on the gpsimd software-DGE path), pipelined per 128-column block so
    hat building overlaps descriptor generation.
  - the 4 scatter-add histograms (iwe / iwe_ts x pol0 / all) are computed as
    sums of rank-1 outer products of bilinear "hat" row/col vectors, using
    fp8e4 DoubleRow matmuls (2 event-chunks contracted per pass at 0.5
    cycles/row -> 4x the bf16 streaming rate):
        hat'(d) = min(|d|,1) - 1 = -relu(1 - |d|)   (negative form, one
        tensor_scalar pass from |d|; products of two negative hats are
        positive so no sign fixup is needed)
        |d| built from d = k - w via a uint16 bitwise-and (sign-bit clear),
        both passes run in the DVE 4x perf mode.
  - PSUM has only 8 banks = one tref's 8 half-images, so each 128-column
    block runs tref=1 then tref=0 through the same banks, flushing into
    SBUF f32 accumulators between trefs.
  - per-tref loss terms and charbonnier smoothness reduced on device to
    per-partition partial sums; host sums 8 cores' partials.
"""

import sys

import numpy as np

sys.path.insert(0, "/opt/trn_rl_repo")

import concourse.bass as bass
import concourse.bacc as bacc
import concourse.tile as tile
from concourse import mybir
from concourse.alu_op_type import AluOpType as Alu

F32 = mybir.dt.float32
F16 = mybir.dt.float16
U16 = mybir.dt.uint16
FP8 = mybir.dt.float8e4
I32 = mybir.dt.int32
AF = mybir.ActivationFunctionType
DR = mybir.MatmulPerfMode.DoubleRow

H, W = 256, 336
SCALE = 336.0
REG_WEIGHT = 0.001
B = 8
N_FULL = 262144
P = 128

LOSS_COLS = list(range(8))        # 2 trefs x 2 pols x 2 y-halves
SMOOTH_COLS = list(range(8, 14))  # 2 ch x {dxA, dxB, dy}

VARIANTS = ("all", "p0", "tsall", "tsp0")


def build_nc(n_events=N_FULL, blk_cols=64, hyt_act=True, hxp_act=True):
    """Build the SPMD Bass program for one core / one batch element."""
    cols = n_events // P          # events laid out as [128, cols]
    n_blocks = cols // blk_cols
    assert blk_cols % 2 == 0
    blk_pairs = blk_cols // 2

    nc = bacc.Bacc("TRN2", target_bir_lowering=False, debug=False, num_devices=8)

    ev_t = nc.declare_dram_parameter("ev_t", [4, n_events], F32, isOutput=False)
    flow_i = nc.declare_dram_parameter("flow_i", [H * W, 2], F32, isOutput=False)
    flow_r = nc.declare_dram_parameter("flow_r", [2, H, W], F32, isOutput=False)
    out = nc.declare_dram_parameter("out", [P, 16], F32, isOutput=True)

    with tile.TileContext(nc) as tc:
        with (
            tc.tile_pool(name="persist", bufs=1) as persist,
            tc.tile_pool(name="acc", bufs=1) as accp,
            tc.tile_pool(name="blk", bufs=3) as blkp,
            tc.tile_pool(name="fgp", bufs=4) as fgp,
            tc.tile_pool(name="pair", bufs=8) as pairp,
            tc.tile_pool(name="small", bufs=1) as small,
            tc.tile_pool(name="psum", bufs=1, space="PSUM") as psum,
        ):
            # ---------------- constants ----------------
            iota_i = small.tile([P, W], I32, tag="iota_i")
            nc.gpsimd.iota(iota_i[:], pattern=[[1, W]], base=0, channel_multiplier=0)
            iota_y = small.tile([P, H], F16, tag="iota_y")
            iota_x = small.tile([P, W], F16, tag="iota_x")
            nc.vector.tensor_copy(iota_y[:], iota_i[:, :H])
            nc.vector.tensor_copy(iota_x[:], iota_i[:])

            out_t = small.tile([P, 16], F32, tag="out_t")
            nc.vector.memset(out_t[:], 0.0)
            c1em6 = small.tile([P, 1], F32, tag="c1em6")
            nc.vector.memset(c1em6[:], 1e-6)

            # ---------------- load event scalars ----------------
            def load_row(r, tag):
                t = persist.tile([P, cols], F32, tag=tag)
                nc.sync.dma_start(out=t[:], in_=ev_t[r].rearrange("(p c) -> p c", p=P))
                return t

            ts_t = load_row(0, "ts")
            ys_t = load_row(1, "ys")
            xs_t = load_row(2, "xs")
            pm_t = load_row(3, "pm")

            # per-event scalars: tg (per tref) lives in ts_t / tg0_t
            tg0_t = persist.tile([P, cols], F32, tag="tg0")
            nc.vector.tensor_scalar(tg0_t[:], ts_t[:], -1.0, 1.0, Alu.mult, Alu.add)

            pixi = persist.tile([P, cols], I32, tag="pixi")
            nc.vector.scalar_tensor_tensor(
                pixi[:], ys_t[:], float(W), xs_t[:], Alu.mult, Alu.add
            )

            # ---------------- charbonnier smoothness ----------------
            smooth_scratch = small.tile([P, 672], F32, tag="smooth_scratch")
            for ch in range(2):
                Fc = small.tile([P, 2 * W], F32, tag="Fc")
                nc.sync.dma_start(
                    out=Fc[:], in_=flow_r[ch].rearrange("(p r) w -> p (r w)", r=2)
                )
                Fs = small.tile([P - 1, 2 * W], F32, tag="Fs")
                nc.sync.dma_start(
                    out=Fs[:],
                    in_=flow_r[ch, 1 : 2 * P - 1, :].rearrange("(p r) w -> p (r w)", r=2),
                )
                dxa = small.tile([P, W], F32, tag="dxa")
                nc.vector.tensor_tensor(dxa[:], Fc[:, 0:W], Fc[:, W : 2 * W], Alu.subtract)
                nc.vector.scalar_tensor_tensor(
                    smooth_scratch[:, 0:W], dxa[:], 0.0, dxa[:], Alu.add, Alu.mult
                )
                nc.scalar.activation(
                    smooth_scratch[:, 0:W], smooth_scratch[:, 0:W], AF.Sqrt,
                    bias=c1em6[:], scale=1.0,
                    accum_out=out_t[:, 8 + 3 * ch : 9 + 3 * ch],
                )
                dxb = small.tile([P - 1, W], F32, tag="dxb")
                nc.vector.tensor_tensor(
                    dxb[:], Fc[: P - 1, W : 2 * W], Fs[:, W : 2 * W], Alu.subtract
                )
                nc.vector.scalar_tensor_tensor(
                    smooth_scratch[: P - 1, 0:W], dxb[:], 0.0, dxb[:], Alu.add, Alu.mult
                )
                nc.scalar.activation(
                    smooth_scratch[: P - 1, 0:W], smooth_scratch[: P - 1, 0:W], AF.Sqrt,
                    bias=c1em6[: P - 1], scale=1.0,
                    accum_out=out_t[: P - 1, 9 + 3 * ch : 10 + 3 * ch],
                )
                dy = small.tile([P, 2 * (W - 1)], F32, tag="dy_s")
                src_a = Fc[:].rearrange("p (r w) -> p r w", r=2)[:, :, 0 : W - 1]
                src_b = Fc[:].rearrange("p (r w) -> p r w", r=2)[:, :, 1:W]
                nc.vector.tensor_tensor(
                    dy[:].rearrange("p (r w) -> p r w", r=2), src_a, src_b, Alu.subtract
                )
                nc.vector.scalar_tensor_tensor(
                    smooth_scratch[:, 0 : 2 * (W - 1)], dy[:], 0.0, dy[:], Alu.add, Alu.mult
                )
                nc.scalar.activation(
                    smooth_scratch[:, 0 : 2 * (W - 1)], smooth_scratch[:, 0 : 2 * (W - 1)],
                    AF.Sqrt, bias=c1em6[:], scale=1.0,
                    accum_out=out_t[:, 10 + 3 * ch : 11 + 3 * ch],
                )

            # ---------------- SBUF image accumulators ----------------
            # acc[t][(v, h)] : [P, W] f32, positive-true images
            acc = {}
            for t in range(2):
                for v in VARIANTS:
                    for h in range(2):
                        acc[(t, v, h)] = accp.tile(
                            [P, W], F32, tag=f"acc_{t}_{v}_{h}", name=f"acc_{t}_{v}_{h}"
                        )

            # psum accumulators, shared between trefs (reused per block)
            ps = {}
            for v in VARIANTS:
                for h in range(2):
                    ps[(v, h)] = psum.tile([P, W], F32, tag=f"ps_{v}_{h}", name=f"ps_{v}_{h}")

            # ---------------- main block loop ----------------
            for blk in range(n_blocks):
                c0 = blk * blk_cols
                csl = slice(c0, c0 + blk_cols)

                # flow gather for this block: one indirect DMA per column
                fg = fgp.tile([P, 2 * blk_cols], F32, tag="fg")
                for j in range(blk_cols):
                    nc.gpsimd.indirect_dma_start(
                        out=fg[:, 2 * j : 2 * j + 2],
                        out_offset=None,
                        in_=flow_i[:],
                        in_offset=bass.IndirectOffsetOnAxis(
                            ap=pixi[:, c0 + j : c0 + j + 1], axis=0
                        ),
                    )
                fy = fg[:].rearrange("p (c two) -> p c two", two=2)[:, :, 0]
                fx = fg[:].rearrange("p (c two) -> p c two", two=2)[:, :, 1]

                # warped coords for both trefs (block slices); scalar operands
                # must stay f32 (ISA: PTR scalars are read as float32)
                wyx = {}
                m1 = blkp.tile([P, blk_cols], F32, tag="m1")
                for t, tref in ((0, 1.0), (1, 0.0)):
                    wy = blkp.tile([P, blk_cols], F32, tag=f"wy{t}")
                    wx = blkp.tile([P, blk_cols], F32, tag=f"wx{t}")
                    nc.vector.scalar_tensor_tensor(
                        m1[:], ts_t[:, csl], float(tref), fy, Alu.subtract, Alu.mult
                    )
                    nc.vector.scalar_tensor_tensor(
                        wy[:], m1[:], -SCALE, ys_t[:, csl], Alu.mult, Alu.add
                    )
                    nc.vector.scalar_tensor_tensor(
                        m1[:], ts_t[:, csl], float(tref), fx, Alu.subtract, Alu.mult
                    )
                    nc.vector.scalar_tensor_tensor(
                        wx[:], m1[:], -SCALE, xs_t[:, csl], Alu.mult, Alu.add
                    )
                    wyx[t] = (wy, wx)

                for t in range(2):
                    wy, wx = wyx[t]
                    tg = ts_t if t == 0 else tg0_t
                    for p in range(blk_pairs):
                        cA = c0 + 2 * p
                        # pair tiles: two DoubleRow slots each
                        d_y = pairp.tile([P, 2 * H], F16, tag="d_y")
                        d_x = pairp.tile([P, 2 * W], F16, tag="d_x")
                        hy = pairp.tile([P, 2 * H], FP8, tag="hy")
                        hyt = pairp.tile([P, 2 * H], FP8, tag="hyt")
                        hx = pairp.tile([P, 2 * W], FP8, tag="hx")
                        hxp = pairp.tile([P, 2 * W], FP8, tag="hxp")
                        for j in range(2):
                            c = 2 * p + j
                            sy = slice(j * H, (j + 1) * H)
                            sx = slice(j * W, (j + 1) * W)
                            # d = k - w   (f16)
                            nc.vector.tensor_scalar(
                                d_y[:, sy], iota_y[:], wy[:, c : c + 1], None, Alu.subtract
                            )
                            nc.vector.tensor_scalar(
                                d_x[:, sx], iota_x[:], wx[:, c : c + 1], None, Alu.subtract
                            )
                        # |d| via sign-bit clear (u16 view, whole pair)
                        nc.vector.tensor_scalar(
                            d_y[:].bitcast(U16), d_y[:].bitcast(U16), 0x7FFF, None,
                            Alu.bitwise_and,
                        )
                        nc.vector.tensor_scalar(
                            d_x[:].bitcast(U16), d_x[:].bitcast(U16), 0x7FFF, None,
                            Alu.bitwise_and,
                        )
                        # hat' = min(a,1) - 1  (negative hats, fp8, whole pair)
                        nc.vector.tensor_scalar(
                            hy[:], d_y[:], 1.0, 1.0, Alu.min, Alu.subtract
                        )
                        nc.vector.tensor_scalar(
                            hx[:], d_x[:], 1.0, 1.0, Alu.min, Alu.subtract
                        )
                        # variant mults (per slot: per-event scalar) on ACT
                        for j in range(2):
                            cg = cA + j
                            sy = slice(j * H, (j + 1) * H)
                            sx = slice(j * W, (j + 1) * W)
                            if hyt_act:
                                nc.scalar.activation(
                                    hyt[:, sy], hy[:, sy], AF.Copy,
                                    bias=0.0, scale=tg[:, cg : cg + 1],
                                )
                            else:
                                nc.vector.tensor_scalar(
                                    hyt[:, sy], hy[:, sy], tg[:, cg : cg + 1], None,
                                    Alu.mult,
                                )
                            if hxp_act:
                                nc.scalar.activation(
                                    hxp[:, sx], hx[:, sx], AF.Copy,
                                    bias=0.0, scale=pm_t[:, cg : cg + 1],
                                )
                            else:
                                nc.vector.tensor_scalar(
                                    hxp[:, sx], hx[:, sx], pm_t[:, cg : cg + 1], None,
                                    Alu.mult,
                                )
                        # 8 DoubleRow matmuls: contract both slots (256 events)
                        start = p == 0
                        stop = p == blk_pairs - 1
                        hyr = hy[:].rearrange("p (two m) -> p two m", two=2)
                        hytr = hyt[:].rearrange("p (two m) -> p two m", two=2)
                        hxr = hx[:].rearrange("p (two n) -> p two n", two=2)
                        hxpr = hxp[:].rearrange("p (two n) -> p two n", two=2)
                        for h in range(2):
                            sh = slice(h * P, (h + 1) * P)
                            for v, lhs, rhs in (
                                ("all", hyr, hxr),
                                ("p0", hyr, hxpr),
                                ("tsall", hytr, hxr),
                                ("tsp0", hytr, hxpr),
                            ):
                                nc.tensor.matmul(
                                    ps[(v, h)][:],
                                    lhsT=lhs[:, :, sh],
                                    rhs=rhs,
                                    start=start,
                                    stop=stop,
                                    perf_mode=DR,
                                )
                    # flush psum -> SBUF accumulators (DVE)
                    for v in VARIANTS:
                        for h in range(2):
                            if blk == 0:
                                nc.vector.tensor_copy(
                                    acc[(t, v, h)][:], ps[(v, h)][:]
                                )
                            else:
                                nc.vector.tensor_tensor(
                                    acc[(t, v, h)][:], acc[(t, v, h)][:],
                                    ps[(v, h)][:], Alu.add,
                                )

            # ---------------- loss terms from accumulated images ----------------
            for t in range(2):
                for h in range(2):
                    s_all = acc[(t, "all", h)]
                    s_p0 = acc[(t, "p0", h)]
                    t_all = acc[(t, "tsall", h)]
                    t_p0 = acc[(t, "tsp0", h)]
                    # pol1 = all - p0 (in place into the "all" tiles)
                    nc.vector.tensor_tensor(s_all[:], s_all[:], s_p0[:], Alu.subtract)
                    nc.vector.tensor_tensor(t_all[:], t_all[:], t_p0[:], Alu.subtract)
                    for pi, (S_img, T_img) in enumerate(((s_p0, t_p0), (s_all, t_all))):
                        r = small.tile([P, W], F32, tag="recip")
                        nc.vector.tensor_scalar(r[:], S_img[:], 1e-9, None, Alu.add)
                        nc.vector.reciprocal(r[:], r[:])
                        q = small.tile([P, W], F32, tag="q")
                        nc.vector.tensor_tensor(q[:], T_img[:], r[:], Alu.mult)
                        col = 4 * t + 2 * h + pi
                        nc.vector.scalar_tensor_tensor(
                            r[:], q[:], 0.0, q[:], Alu.add, Alu.mult,
                            accum_out=out_t[:, col : col + 1],
                        )

            nc.sync.dma_start(out=out[:], in_=out_t[:])

    nc.finalize()
    return nc


# ---------------------------------------------------------------------------
# host-side wrapper
# ---------------------------------------------------------------------------

_CACHED = {}


def _get_nc():
    key = "full"
    if key not in _CACHED:
        _CACHED[key] = build_nc()
    return _CACHED[key]


def prep_core_inputs(flow_b, ev_b, pm_b):
    """Per-batch-element host prep: pure re-layout (sharding), no math beyond layout."""
    n = ev_b.shape[0]
    ev_t = np.empty((4, n), dtype=np.float32)
    ev_t[0] = ev_b[:, 0]
    ev_t[1] = ev_b[:, 1]
    ev_t[2] = ev_b[:, 2]
    ev_t[3] = pm_b[:, 0]
    flow_i = np.ascontiguousarray(
        np.stack([flow_b[1].reshape(-1), flow_b[0].reshape(-1)], axis=-1)
    ).astype(np.float32)
    flow_r = np.ascontiguousarray(flow_b).astype(np.float32)
    return {"ev_t": ev_t, "flow_i": flow_i, "flow_r": flow_r}


def finish(outs):
    """Combine per-core partials into the scalar loss."""
    total = np.float64(0.0)
    for o in outs:
        o = o.astype(np.float64)
        total += o[:, LOSS_COLS].sum() + REG_WEIGHT * o[:, SMOOTH_COLS].sum()
    return np.float32(total)


def kernel(flow, event_list, pol_mask):
    from concourse.bass_utils import run_bass_kernel_spmd

    flow = np.asarray(flow)
    event_list = np.asarray(event_list)
    pol_mask = np.asarray(pol_mask)
    nc = _get_nc()
    in_maps = [
        prep_core_inputs(flow[b], event_list[b], pol_mask[b]) for b in range(B)
    ]
    res = run_bass_kernel_spmd(nc, in_maps, list(range(B)))
    outs = [res.results[b]["out"] for b in range(B)]
    return finish(outs)


if __name__ == "__main__":
    rng = np.random.default_rng(0)
    flow = (0.05 * rng.standard_normal((B, 2, H, W))).astype(np.float32)
    ys = rng.integers(0, H, (B, N_FULL)).astype(np.float32)
    xs = rng.integers(0, W, (B, N_FULL)).astype(np.float32)
    ts = rng.random((B, N_FULL), dtype=np.float32)
    pol = rng.integers(0, 2, (B, N_FULL))
    ev = np.stack([ts, ys, xs, pol * 2.0 - 1.0], axis=-1).astype(np.float32)
    pm = np.stack([(pol == 1), (pol == 0)], axis=-1).astype(np.float32)
    print(kernel(flow, ev, pm))
